# revision 1
# baseline (speedup 1.0000x reference)
"""Trainium2 Bass kernel for nn_MultiHeadContrastive (two-head contrastive loss).

Strategy (8 NeuronCores, two SPMD launches, no collectives):

  Launch 1 (MLP): rows of roi_feats are sorted by group
  (anchor / fg-low-iou / bg / ignore) on the host and sharded contiguously,
  1024 rows per core.  Each core computes both projection heads for its rows
  (transposed layout zT = [d, rows], fp32) via TensorE and returns the raw
  (pre-normalization) embeddings.

  Host: gathers the 8 z shards, L2-normalizes rows in float64, casts fp32.

  Launch 2 (SIM): every core receives the full normalized key matrices
  zT_fg [64, 8192], zT_cls [128, 8192] plus its private slice of anchor
  columns.  For each 128-anchor block it computes sim^T = anchors x keys via
  TensorE into PSUM (raw dot products), then ScalarE evaluates
  exp(dot / TAU) in place with accum_out producing per-anchor row sums per
  key range.  Because rows were sorted, the three masked sums the losses
  need (all keys / fg keys / non-ignored keys) are plain prefix-range sums,
  so no mask tensors and no second pass over the N^2 matrix exist at all.
  Anchors are restricted to rows with label>0, not ignored, and iou>0.5 —
  every other row contributes exactly zero to the weighted losses.

  Host: subtracts the self-similarity terms, computes the class-positive
  term of the SupCon loss from per-class sums of z (an O(N*D) computation),
  applies logs/weights in float64, and returns the 2-element loss vector.
"""

import math
import os

import numpy as np

import concourse.bacc as bacc
import concourse.mybir as mybir
import concourse.tile as tile
from concourse.bass_utils import run_bass_kernel_spmd

N_CORES = 8
N, C = 8192, 1024
HID, DF, DC = 256, 64, 128
TAU = 0.2
EPS = 1e-8
EPS12 = 1e-12
IOU_THRESHOLD = 0.5

F32 = mybir.dt.float32
F32R = mybir.dt.float32r
ACT = mybir.ActivationFunctionType
AX = mybir.AxisListType

# Introspection for test.py: BassKernelResults of the two launches.
LAST_RESULTS = []

# Built Bass modules are pure functions of their config; cache across calls.
_NC_CACHE = {}


def _build_mlp_nc():
    """Launch 1: per-core MLP producing raw zT for both heads."""
    R = N // N_CORES  # rows per core
    KC = C // 128     # feature chunks
    KH = HID // 128   # hidden chunks
    RB = 512          # moving free dim per matmul (fp32 limit)
    NR = R // RB

    nc = bacc.Bacc(trn_type="TRN2", num_devices=N_CORES, debug=False)
    xT = nc.dram_tensor("xT", [C, R], F32R, kind="ExternalInput")
    w1fT = nc.dram_tensor("w1fT", [C, HID], F32R, kind="ExternalInput")
    w2fT = nc.dram_tensor("w2fT", [HID, DF], F32R, kind="ExternalInput")
    w1cT = nc.dram_tensor("w1cT", [C, HID], F32R, kind="ExternalInput")
    w2cT = nc.dram_tensor("w2cT", [HID, DC], F32R, kind="ExternalInput")
    b1f = nc.dram_tensor("b1f", [HID, 1], F32, kind="ExternalInput")
    b2f = nc.dram_tensor("b2f", [DF, 1], F32, kind="ExternalInput")
    b1c = nc.dram_tensor("b1c", [HID, 1], F32, kind="ExternalInput")
    b2c = nc.dram_tensor("b2c", [DC, 1], F32, kind="ExternalInput")
    zf = nc.dram_tensor("zf", [DF, R], F32, kind="ExternalOutput")
    zc = nc.dram_tensor("zc", [DC, R], F32, kind="ExternalOutput")

    with tile.TileContext(nc) as tc:
        with (
            tc.tile_pool(name="cst", bufs=1) as cst,
            tc.tile_pool(name="hb", bufs=2) as hb,
            tc.tile_pool(name="zb", bufs=2) as zb,
            tc.tile_pool(name="ps", bufs=1, space="PSUM") as ps,
        ):
            # per-chunk tiles so matmuls on chunk k only wait for chunk k's
            # DMA; interleave x and w1 chunk loads so the k-th chain step has
            # both operands as early as possible.
            heads = (
                ("f", w1fT, w2fT, b1f, b2f, DF, zf),
                ("c", w1cT, w2cT, b1c, b2c, DC, zc),
            )
            xt_t = []
            w1_t = {"f": [], "c": []}
            for k in range(KC):
                t = cst.tile([128, R], F32R, tag=f"xt{k}", name=f"xt{k}")
                nc.sync.dma_start(out=t[:, :], in_=xT[k * 128:(k + 1) * 128, :])
                xt_t.append(t)
                for hname, w1d, *_ in heads:
                    tw = cst.tile([128, HID], F32R, tag=f"w1{hname}{k}")
                    nc.sync.dma_start(out=tw[:, :], in_=w1d[k * 128:(k + 1) * 128, :])
                    w1_t[hname].append(tw)

            for hi, (hname, w1d, w2d, b1d, b2d, d, zout) in enumerate(heads):
                w2t = cst.tile([128, KH, d], F32R, tag=f"w2{hname}")
                for h in range(KH):
                    nc.sync.dma_start(out=w2t[:, h, :], in_=w2d[h * 128:(h + 1) * 128, :])
                b1t = cst.tile([128, KH], F32, tag=f"b1{hname}")
                for h in range(KH):
                    nc.sync.dma_start(out=b1t[:, h:h + 1], in_=b1d[h * 128:(h + 1) * 128, :])
                b2t = cst.tile([d, 1], F32, tag=f"b2{hname}")
                nc.sync.dma_start(out=b2t[:, :], in_=b2d[:, :])

                hsb = hb.tile([128, KH, R], F32R, tag=f"h{hname}")
                # all four (h, r) accumulation chains advance together as each
                # xT chunk lands, so PE finishes ~right after the last chunk
                hps = {}
                for h in range(KH):
                    for r in range(NR):
                        pidx = hi * 4 + h * NR + r
                        hps[(h, r)] = ps.tile(
                            [128, RB], F32, tag=f"p{pidx}", name=f"hp{pidx}"
                        )
                for k in range(KC):
                    for (h, r), hp in hps.items():
                        nc.tensor.matmul(
                            out=hp[:, :],
                            lhsT=w1_t[hname][k][:, h * 128:(h + 1) * 128],
                            rhs=xt_t[k][:, r * RB:(r + 1) * RB],
                            start=(k == 0),
                            stop=(k == KC - 1),
                        )
                for r in range(NR):
                    for h in range(KH):
                        # hT = relu(w1 @ xT + b1) fused on DVE (also rounds
                        # to fp32r for the next matmul); b1 is per-partition.
                        nc.vector.tensor_scalar(
                            out=hsb[:, h, r * RB:(r + 1) * RB],
                            in0=hps[(h, r)][:, :],
                            scalar1=b1t[:, h:h + 1],
                            scalar2=0.0,
                            op0=mybir.AluOpType.add,
                            op1=mybir.AluOpType.max,
                        )
                    # reuse the bank of the (h0, r) chain this head just
                    # drained via its relu — PSUM stays within 8 banks
                    zp = ps.tile([128, RB], F32, tag=f"p{hi * 4 + r}", name=f"zp{hi}{r}")
                    for h in range(KH):
                        nc.tensor.matmul(
                            out=zp[:d, :],
                            lhsT=w2t[:, h, :],
                            rhs=hsb[:, h, r * RB:(r + 1) * RB],
                            start=(h == 0),
                            stop=(h == KH - 1),
                        )
                    zt = zb.tile([d, RB], F32, tag=f"z{hname}")
                    nc.scalar.activation(
                        out=zt[:, :],
                        in_=zp[:d, :],
                        func=ACT.Identity,
                        bias=b2t[:, 0:1],
                        scale=1.0,
                    )
                    nc.sync.dma_start(out=zout[:, r * RB:(r + 1) * RB], in_=zt[:, :])
    nc.compile()
    return nc


def _build_sim_nc(n_fg, n_valid, nblk):
    """Launch 2: per-anchor-block sim matmuls + fused exp/prefix-range sums.

    Returns (nc, numer_cols, nfgcols, ngc): stats output columns are
      0: sum_{all keys} exp(sim/TAU)
      1: sum_{keys < n_fg} exp(sim/TAU)
      2: sum_{keys < n_valid} exp(sim/TAU)
    (all including the anchor's self term, subtracted on the host).
    """
    A = nblk * 128
    G = 2048
    NGF = N // G
    NGC = (n_valid + G - 1) // G

    nc = bacc.Bacc(trn_type="TRN2", num_devices=N_CORES, debug=False)
    zfk = nc.dram_tensor("zfk", [DF, N], F32R, kind="ExternalInput")
    zck = nc.dram_tensor("zck", [DC, N], F32R, kind="ExternalInput")
    zfa = nc.dram_tensor("zfa", [DF, A], F32R, kind="ExternalInput")
    zca = nc.dram_tensor("zca", [DC, A], F32R, kind="ExternalInput")
    stats = nc.dram_tensor("stats", [nblk, 128, 3], F32, kind="ExternalOutput")

    # fg-head exp/accum pieces: split each 2048-key group at the n_fg
    # boundary so masked sums become plain column-range selections.
    fg_pieces = []  # (group, c0, c1, col)
    col = 0
    numer_cols = 0
    for g in range(NGF):
        lo, hi = g * G, (g + 1) * G
        cuts = [lo, n_fg, hi] if lo < n_fg < hi else [lo, hi]
        for a0, a1 in zip(cuts[:-1], cuts[1:]):
            fg_pieces.append((g, a0 - lo, a1 - lo, col))
            if a1 <= n_fg:
                numer_cols = col + 1
            col += 1
    nfgcols = col

    with tile.TileContext(nc) as tc:
        with (
            tc.tile_pool(name="keys", bufs=1) as keys,
            tc.tile_pool(name="anch", bufs=1) as anch,
            tc.tile_pool(name="st", bufs=3) as st,
            tc.tile_pool(name="ps", bufs=2, space="PSUM") as ps,
        ):
            # fg anchors + first fg key group gate the very first matmul:
            # issue them before anything else; cls anchors/keys are not
            # needed until the first anchor block's cls phase (~7us in).
            zfa_t = anch.tile([DF, A], F32R, tag="zfa")
            nc.sync.dma_start(out=zfa_t[:, :], in_=zfa[:, :])
            # warm up the ACT exp table load while DMAs stream
            wu = st.tile([1, 8], F32, tag="wu")
            nc.vector.memset(wu[:, :], 0.0)
            nc.scalar.activation(out=wu[:, :], in_=wu[:, :], func=ACT.Exp, scale=1.0)
            zfk_t = [None] * NGF
            zck_t = [None] * NGC
            zca_t = None

            def _load_f(g):
                t = keys.tile([DF, G], F32R, tag=f"zfk{g}", name=f"zfk{g}")
                nc.sync.dma_start(out=t[:, :], in_=zfk[:, g * G:(g + 1) * G])
                zfk_t[g] = t

            def _load_c(g):
                # load the full group (cols past n_valid are real rows too);
                # only the exp/accum below is range-restricted
                t = keys.tile([DC, G], F32R, tag=f"zck{g}", name=f"zck{g}")
                nc.sync.dma_start(out=t[:, :], in_=zck[:, g * G:(g + 1) * G])
                zck_t[g] = t

            _load_f(0)
            _load_f(1)
            zca_t = anch.tile([DC, A], F32R, tag="zca")
            nc.sync.dma_start(out=zca_t[:, :], in_=zca[:, :])
            if NGC > 0:
                _load_c(0)
            _load_f(2)
            _load_f(3)
            for g in range(1, NGC):
                _load_c(g)

            for ab in range(nblk):
                lf = zfa_t[:, ab * 128:(ab + 1) * 128]
                lc = zca_t[:, ab * 128:(ab + 1) * 128]
                sf = st.tile([128, nfgcols], F32, tag="sf")
                sc = st.tile([128, NGC], F32, tag="sc")
                for g in range(NGF):
                    p = ps.tile([128, G], F32, tag="ps")
                    for kk in range(G // 512):
                        nc.tensor.matmul(
                            out=p[:, kk * 512:(kk + 1) * 512],
                            lhsT=lf,
                            rhs=zfk_t[g][:, kk * 512:(kk + 1) * 512],
                            start=True,
                            stop=True,
                        )
                    for gg, c0, c1, pcol in fg_pieces:
                        if gg != g:
                            continue
                        nc.scalar.activation(
                            out=p[:, c0:c1],
                            in_=p[:, c0:c1],
                            func=ACT.Exp,
                            scale=1.0 / TAU,
                            accum_out=sf[:, pcol:pcol + 1],
                        )
                for g in range(NGC):
                    klim = min(G, n_valid - g * G)
                    p = ps.tile([128, G], F32, tag="ps")
                    # full-width matmuls (fp32r needs large even free dims);
                    # only [0:klim] is exp'd/accumulated below
                    for kk in range(G // 512):
                        if kk * 512 >= klim:
                            break
                        nc.tensor.matmul(
                            out=p[:, kk * 512:(kk + 1) * 512],
                            lhsT=lc,
                            rhs=zck_t[g][:, kk * 512:(kk + 1) * 512],
                            start=True,
                            stop=True,
                        )
                    nc.scalar.activation(
                        out=p[:, 0:klim],
                        in_=p[:, 0:klim],
                        func=ACT.Exp,
                        scale=1.0 / TAU,
                        accum_out=sc[:, g:g + 1],
                    )
                o3 = st.tile([128, 3], F32, tag="o3")
                nc.vector.reduce_sum(out=o3[:, 0:1], in_=sf[:, 0:nfgcols], axis=AX.X)
                nc.vector.reduce_sum(out=o3[:, 1:2], in_=sf[:, 0:numer_cols], axis=AX.X)
                nc.vector.reduce_sum(out=o3[:, 2:3], in_=sc[:, 0:NGC], axis=AX.X)
                nc.sync.dma_start(out=stats[ab, :, :], in_=o3[:, :])
    nc.compile()
    return nc


LAST_TIMES = []


def _run(nc, in_maps, out_names):
    import time as _time

    if os.environ.get("CC_BASS_SIM") == "1":
        from concourse import bass_interp

        results = []
        for m in range(N_CORES):
            sim = bass_interp.CoreSim(nc, core_id=m)
            for k, v in in_maps[m].items():
                sim.tensor(k)[:] = v
            if nc.partition_id_tensor is not None:
                sim.tensor(nc.partition_id_tensor.name)[:] = np.array(
                    [[m]], dtype=np.uint32
                )
            sim.simulate()
            results.append(
                {name: np.array(sim.mem_tensor(name)) for name in out_names}
            )
        return results
    t0 = _time.monotonic()
    res = run_bass_kernel_spmd(nc, in_maps, core_ids=list(range(N_CORES)))
    LAST_TIMES.append(_time.monotonic() - t0)
    LAST_RESULTS.append(res)
    return res.results


def kernel(**inputs):
    global LAST_RESULTS, LAST_TIMES
    LAST_RESULTS = []
    LAST_TIMES = []

    roi = np.ascontiguousarray(np.asarray(inputs["roi_feats"], dtype=np.float32))
    labels = np.asarray(inputs["labels"]).astype(np.int64)
    ious = np.asarray(inputs["ious"], dtype=np.float32)
    w1f = np.asarray(inputs["w1f"], dtype=np.float32)
    b1f = np.asarray(inputs["b1f"], dtype=np.float32)
    w2f = np.asarray(inputs["w2f"], dtype=np.float32)
    b2f = np.asarray(inputs["b2f"], dtype=np.float32)
    w1c = np.asarray(inputs["w1c"], dtype=np.float32)
    b1c = np.asarray(inputs["b1c"], dtype=np.float32)
    w2c = np.asarray(inputs["w2c"], dtype=np.float32)
    b2c = np.asarray(inputs["b2c"], dtype=np.float32)
    assert roi.shape == (N, C)

    ign = labels == -1
    fg = (labels > 0) & ~ign
    bg = (labels == 0) & ~ign
    anc = fg & (ious > IOU_THRESHOLD)

    perm = np.concatenate(
        [
            np.where(anc)[0],
            np.where(fg & ~anc)[0],
            np.where(bg)[0],
            np.where(ign)[0],
        ]
    )
    n_A = int(anc.sum())
    n_fg = int(fg.sum())
    n_valid = n_fg + int(bg.sum())

    if n_A == 0:
        return np.zeros(2, dtype=np.float32)

    x_s = roi[perm]
    labels_s = labels[perm]
    ious_s = ious[perm].astype(np.float64)

    # ---------------- launch 1: MLP ----------------
    if "mlp" not in _NC_CACHE:
        _NC_CACHE["mlp"] = _build_mlp_nc()
    nc1 = _NC_CACHE["mlp"]
    xT = np.ascontiguousarray(x_s.T)  # [C, N]
    R = N // N_CORES
    shared1 = {
        "w1fT": np.ascontiguousarray(w1f.T),
        "w2fT": np.ascontiguousarray(w2f.T),
        "w1cT": np.ascontiguousarray(w1c.T),
        "w2cT": np.ascontiguousarray(w2c.T),
        "b1f": b1f.reshape(HID, 1).copy(),
        "b2f": b2f.reshape(DF, 1).copy(),
        "b1c": b1c.reshape(HID, 1).copy(),
        "b2c": b2c.reshape(DC, 1).copy(),
    }
    in_maps1 = [
        {"xT": np.ascontiguousarray(xT[:, m * R:(m + 1) * R]), **shared1}
        for m in range(N_CORES)
    ]
    res1 = _run(nc1, in_maps1, ["zf", "zc"])

    zfT_raw = np.concatenate([r["zf"] for r in res1], axis=1)  # [DF, N]
    zcT_raw = np.concatenate([r["zc"] for r in res1], axis=1)  # [DC, N]

    # ---------------- host: normalize in float64, cast fp32 ----------------
    def _normalize(zT_raw):
        z = zT_raw.T.astype(np.float64)  # [N, d]
        nrm = np.sqrt(np.sum(z * z, axis=1, keepdims=True))
        zn = z / np.maximum(nrm, EPS)
        return zn.astype(np.float32)

    zfn = _normalize(zfT_raw)  # [N, DF] fp32, sorted order
    zcn = _normalize(zcT_raw)  # [N, DC]

    # ---------------- launch 2: sims ----------------
    nblk = max(1, math.ceil(math.ceil(n_A / N_CORES) / 128))
    A_pc = nblk * 128
    sim_key = ("sim", n_fg, n_valid, nblk)
    if sim_key not in _NC_CACHE:
        _NC_CACHE[sim_key] = _build_sim_nc(n_fg, n_valid, nblk)
    nc2 = _NC_CACHE[sim_key]

    zfkT = np.ascontiguousarray(zfn.T)  # [DF, N]
    zckT = np.ascontiguousarray(zcn.T)  # [DC, N]
    in_maps2 = []
    for m in range(N_CORES):
        idx = np.minimum(np.arange(m * A_pc, (m + 1) * A_pc), n_A - 1)
        in_maps2.append(
            {
                "zfk": zfkT,
                "zck": zckT,
                "zfa": np.ascontiguousarray(zfkT[:, idx]),
                "zca": np.ascontiguousarray(zckT[:, idx]),
            }
        )
    res2 = _run(nc2, in_maps2, ["stats"])

    # slot s of the concatenated stats covers anchor s; drop padded slots
    stats = np.concatenate([r["stats"].reshape(A_pc, 3) for r in res2], axis=0)
    stats = stats[np.arange(N_CORES * A_pc) < n_A].astype(np.float64)  # [n_A, 3]

    # ---------------- host: final losses in float64 ----------------
    zfa64 = zfn[:n_A].astype(np.float64)
    zca64 = zcn[:n_A].astype(np.float64)
    w_a = ious_s[:n_A]

    selfdot_f = np.sum(zfa64 * zfa64, axis=1)
    selfexp_f = np.exp(selfdot_f / TAU)
    selfdot_c = np.sum(zca64 * zca64, axis=1)
    selfexp_c = np.exp(selfdot_c / TAU)

    # fg/bg loss
    npos_fg = n_fg - 1
    if npos_fg > 0:
        denom = stats[:, 0] - selfexp_f
        numer = stats[:, 1] - selfexp_f
        li = -np.log((numer + EPS) / (denom + EPS))
        loss_fg = np.sum(li * w_a) / (np.sum(w_a) + EPS)
    else:
        loss_fg = 0.0  # num=0, den=EPS -> 0

    # class supcon loss
    lab_valid = labels_s[:n_valid]
    cnt = np.bincount(lab_valid, minlength=21)
    S = np.zeros((21, DC), dtype=np.float64)
    np.add.at(S, lab_valid, zcn[:n_valid].astype(np.float64))
    c_a = labels_s[:n_A]
    n_pos = (cnt[c_a] - 1).astype(np.float64)
    D = stats[:, 2] - selfexp_c
    denom_log = np.log(np.maximum(D, 1e-300))
    sum_pos = (np.einsum("nd,nd->n", zca64, S[c_a]) - selfdot_c) / TAU
    li_c = -(sum_pos - n_pos * denom_log) / np.maximum(n_pos, 1.0)
    valid_c = n_pos > 0
    num2 = np.sum(np.where(valid_c, li_c * w_a, 0.0))
    den2 = np.sum(np.where(valid_c, w_a, 0.0))
    loss_cls = num2 / (den2 + EPS12)

    return np.stack([loss_fg, loss_cls]).astype(np.float32)



# revision 5
# speedup vs baseline: 1.1691x; 1.1691x over previous
"""Trainium2 Bass kernel for nn_MultiHeadContrastive (two-head contrastive loss).

Strategy (8 NeuronCores, two SPMD launches, no collectives):

  Launch 1 (MLP): rows of roi_feats are sorted by group
  (anchor / fg-low-iou / bg / ignore) on the host and sharded contiguously,
  1024 rows per core.  Layer 1 runs in fp8e4(e4m3) with DoubleRow perf mode
  (weights pre-scaled by 64 on the host so fp8 stays in its normal range;
  the scale cancels in the host-side normalization).  Layer 2 runs fp8-DR
  as well (falls back to bf16 if the hidden activations would overflow
  e4m3).  Raw (pre-normalization, pre-b2) embeddings return as bf16; the
  host adds b2, L2-normalizes in float64.

  Launch 2 (SIM): each core owns nblk*128 anchor rows (lhsT) and all N keys
  (rhs, fp8-DR, scaled x16).  Per anchor block, sim rows are built in PSUM
  2048 keys at a time.  exp(sim/TAU) + the three masked range sums are
  computed by TWO engines in parallel: ACT evaluates exact exp with
  accum_out on ~60% of the columns (including every anchor's self column),
  while Pool evaluates a Schraudolph-style exp approximation (affine in the
  exponent, stored as int16 and re-read as bf16) on the rest, which DVE
  then range-sums.  Because rows are sorted, all masked sums are plain
  column-range sums.

  Host: subtracts per-engine-consistent self-similarity terms, computes the
  class-positive term of the SupCon loss from per-class sums of z, applies
  logs/weights in float64, and returns the 2-element loss vector.
"""

import math
import os

import numpy as np
import ml_dtypes

import concourse.bacc as bacc
import concourse.mybir as mybir
import concourse.tile as tile
from concourse.bass_utils import run_bass_kernel_spmd

N_CORES = 8
N, C = 8192, 1024
HID, DF, DC = 256, 64, 128
HID2 = 2 * HID  # both heads' hidden concatenated
TAU = 0.2
EPS = 1e-8
EPS12 = 1e-12
IOU_THRESHOLD = 0.5

F32 = mybir.dt.float32
F32R = mybir.dt.float32r
BF16 = mybir.dt.bfloat16
F8 = mybir.dt.float8e4
I16 = mybir.dt.int16
ACT = mybir.ActivationFunctionType
AX = mybir.AxisListType
ALU = mybir.AluOpType
DR = mybir.MatmulPerfMode.DoubleRow

NP8 = ml_dtypes.float8_e4m3
NPBF = ml_dtypes.bfloat16

W1SCALE = 64.0  # host pre-scale on W1/b1 so fp8 weights are in normal range
ZSCALE = 16.0   # host pre-scale on normalized z before fp8 quantization

# Schraudolph int16-as-bf16 exp approximation:
#   bf16 bits = 128*E + m  (E exponent field, m 7-bit mantissa)
#   value(y)  = 2^(y/128 - 127) * (1 + frac)  ~= 2^(y/128 - 127 + eps)
#   exp(s * k) = 2^(s * k * log2(e))  ->  y = s * (128*k*log2e) + 128*(127 - SIG)
# SIG tuned to zero the mean relative error for uniform mantissa phase.
SCH_SIG = 0.05394  # zero-mean rel. error under float->int16 truncation

# Introspection for test.py: BassKernelResults of the two launches.
LAST_RESULTS = []
LAST_TIMES = []

# Built Bass modules are pure functions of their config; cache across calls.
_NC_CACHE = {}

G = 2048          # key-group (PSUM region) width
NG = N // G       # groups per head


def _schraudolph_consts(psum_scale):
    """y = s_psum * c + b so that bitcast-bf16(int16(y)) ~ exp(s_psum*psum_scale)."""
    c = 128.0 * psum_scale * np.log2(np.e)
    b = 128.0 * (127.0 - SCH_SIG)
    return float(c), float(b)


def _schraudolph_host(s_psum, psum_scale):
    """Replicate the device Pool+DVE exp approximation exactly (fp64 in)."""
    c, b = _schraudolph_consts(psum_scale)
    y = np.asarray(s_psum, np.float64) * c + b
    yi = np.trunc(y).astype(np.int64).astype(np.int16)  # numpy store = truncation
    return yi.view(NPBF).astype(np.float64)


# --------------------------------------------------------------------------
# Launch 1: MLP (both heads, fp8 DoubleRow)
# --------------------------------------------------------------------------
def _build_mlp_nc(layer2_fp8):
    R = N // N_CORES     # 1024 rows per core
    RB = 512             # row block
    NR = R // RB         # 2
    KC4 = C // 256       # 4 DoubleRow k-steps for layer 1
    NH4 = HID2 // 128    # 4 output chunks of 128 hidden units

    nc = bacc.Bacc(trn_type="TRN2", num_devices=N_CORES, debug=False)
    # x_dr[p, k, i, r] = xT[k*256 + i*128 + p, r]
    x_d = nc.dram_tensor("x", [128, KC4, 2, R], F8, kind="ExternalInput")
    # w1_dr[p, k, i, h] = w1T_all[k*256 + i*128 + p, h]  (both heads, x64)
    w1_d = nc.dram_tensor("w1", [128, KC4, 2, HID2], F8, kind="ExternalInput")
    # w2_dr[p, i, d] = w2T_head[i*128 + p, d]; cols 0:DF head f, DF: head c
    w2dt = F8 if layer2_fp8 else BF16
    w2_d = nc.dram_tensor("w2", [128, 2, DF + DC], w2dt, kind="ExternalInput")
    b1_d = nc.dram_tensor("b1", [128, NH4], F32, kind="ExternalInput")
    zf_d = nc.dram_tensor("zf", [DF, R], BF16, kind="ExternalOutput")
    zc_d = nc.dram_tensor("zc", [DC, R], BF16, kind="ExternalOutput")

    hdt = F8 if layer2_fp8 else BF16
    with tile.TileContext(nc) as tc:
        with (
            tc.tile_pool(name="cst", bufs=1) as cst,
            tc.tile_pool(name="hb", bufs=2) as hb,
            tc.tile_pool(name="zb", bufs=2) as zb,
            tc.tile_pool(name="ps", bufs=1, space="PSUM") as ps,
        ):
            # warm the ACT Relu table while DMAs stream
            wu = cst.tile([1, 8], F32, tag="wu")
            nc.vector.memset(wu[:, :], 0.0)
            nc.scalar.activation(out=wu[:, :], in_=wu[:, :], func=ACT.Relu, scale=1.0)

            w1_t = []
            x_t = {}
            # interleave weight chunks and x quarters so k-chains start early
            for k in range(KC4):
                tw = cst.tile([128, 2, HID2], F8, tag=f"w1{k}")
                nc.sync.dma_start(out=tw[:, :, :], in_=w1_d[:, k, :, :])
                w1_t.append(tw)
            b1_t = cst.tile([128, NH4], F32, tag="b1")
            nc.sync.dma_start(out=b1_t[:, :], in_=b1_d[:, :])
            w2_t = cst.tile([128, 2, DF + DC], w2dt, tag="w2")
            nc.sync.dma_start(out=w2_t[:, :, :], in_=w2_d[:, :, :])
            for r in range(NR):
                tx = cst.tile([128, KC4, 2, RB], F8, tag=f"x{r}")
                nc.sync.dma_start(out=tx[:, :, :, :], in_=x_d[:, :, :, r * RB:(r + 1) * RB])
                x_t[r] = tx

            hp = {h4: ps.tile([128, RB], F32, tag=f"hp{h4}", name=f"hp{h4}")
                  for h4 in range(NH4)}
            zp = {0: ps.tile([DF, RB], F32, tag="zpf", name="zpf"),
                  1: ps.tile([DC, RB], F32, tag="zpc", name="zpc")}
            for r in range(NR):
                hsb = hb.tile([128, NH4, RB], hdt, tag=f"h{r}")
                for h4 in range(NH4):
                    for k in range(KC4):
                        nc.tensor.matmul(
                            out=hp[h4][:, :],
                            lhsT=w1_t[k][:, :, h4 * 128:(h4 + 1) * 128],
                            rhs=x_t[r][:, k, :, :],
                            start=(k == 0),
                            stop=(k == KC4 - 1),
                            perf_mode=DR,
                        )
                    # bias + relu + cast; alternate DVE / ACT
                    if h4 % 2 == 0:
                        nc.vector.tensor_scalar(
                            out=hsb[:, h4, :],
                            in0=hp[h4][:, :],
                            scalar1=b1_t[:, h4:h4 + 1],
                            scalar2=0.0,
                            op0=ALU.add,
                            op1=ALU.max,
                        )
                    else:
                        nc.scalar.activation(
                            out=hsb[:, h4, :],
                            in_=hp[h4][:, :],
                            func=ACT.Relu,
                            bias=b1_t[:, h4:h4 + 1],
                            scale=1.0,
                        )
                for hi, (d, zd) in enumerate(((DF, zf_d), (DC, zc_d))):
                    c0 = 0 if hi == 0 else DF
                    if layer2_fp8:
                        nc.tensor.matmul(
                            out=zp[hi][:, :],
                            lhsT=w2_t[:, :, c0:c0 + d],
                            rhs=hsb[:, 2 * hi:2 * hi + 2, :],
                            start=True, stop=True,
                            perf_mode=DR,
                        )
                    else:
                        for kk in range(2):
                            nc.tensor.matmul(
                                out=zp[hi][:, :],
                                lhsT=w2_t[:, kk, c0:c0 + d],
                                rhs=hsb[:, 2 * hi + kk, :],
                                start=(kk == 0), stop=(kk == 1),
                            )
                    zt = zb.tile([d, RB], BF16, tag=f"z{hi}")
                    if hi == 0:
                        nc.vector.tensor_scalar(
                            out=zt[:, :], in0=zp[hi][:, :], scalar1=1.0,
                            scalar2=None, op0=ALU.mult)
                    else:
                        nc.scalar.activation(
                            out=zt[:, :], in_=zp[hi][:, :], func=ACT.Copy,
                            bias=0.0, scale=1.0)
                    nc.sync.dma_start(out=zd[:, r * RB:(r + 1) * RB], in_=zt[:, :])
    nc.compile()
    return nc


# --------------------------------------------------------------------------
# Launch 2: SIM (exp sums, ACT + Pool/DVE split)
# --------------------------------------------------------------------------
def _build_sim_nc(n_fg, n_valid, nblk, a2):
    """Per-core: nblk anchor blocks of 128; per block 8 key groups of 2048.

    fg head: groups 0..3 over all N keys; numer needs [0, n_fg).
    cls head: groups 0..3 over [0, n_valid).
    ACT covers fg G0,G1, cls G0,G1 and the first a2 cols of fg G2 (exact exp,
    accum_out).  Pool covers the rest (Schraudolph int16), DVE range-sums.
    Self columns (global anchor idx < n_A <= 4096) always land in ACT ranges.

    stats columns (per block, fp32):
      0: ACT fg G0+G1 sum            1: ACT cls G0+G1 sum
      2: ACT fg G2[0:a2]             3: DVE fg G2[a2:2048]
      4: DVE fg G3[0:bf]             5: DVE fg G3[bf:2048]
      6: DVE cls G2                  7: DVE cls G3[0:bc]
    where bf = n_fg - 3*G clipped to [0, G]; bc = n_valid - 3*G.
    host: numer_f = s0+s2+s3+s4 - self; denom_f = numer + s5; denom_c = s1+s6+s7 - self
    """
    A = nblk * 128
    assert 3 * G <= n_fg <= 4 * G and 3 * G <= n_valid <= 4 * G
    bf = n_fg - 3 * G
    bc = n_valid - 3 * G
    NSTAT = 8
    act_scale = 1.0 / (ZSCALE * ZSCALE * TAU)
    sch_c, sch_b = _schraudolph_consts(act_scale)

    nc = bacc.Bacc(trn_type="TRN2", num_devices=N_CORES, debug=False)
    zfk_d = nc.dram_tensor("zfk", [32, 2, N], F8, kind="ExternalInput")
    zck_d = nc.dram_tensor("zck", [64, 2, N], F8, kind="ExternalInput")
    zfa_d = nc.dram_tensor("zfa", [32, 2, A], F8, kind="ExternalInput")
    zca_d = nc.dram_tensor("zca", [64, 2, A], F8, kind="ExternalInput")
    st_d = nc.dram_tensor("stats", [nblk, 128, NSTAT], F32, kind="ExternalOutput")

    with tile.TileContext(nc) as tc:
        with (
            tc.tile_pool(name="keys", bufs=1) as keys,
            tc.tile_pool(name="anch", bufs=1) as anch,
            tc.tile_pool(name="sch", bufs=2) as sch,
            tc.tile_pool(name="st", bufs=2) as st,
            tc.tile_pool(name="ps", bufs=1, space="PSUM") as ps,
        ):
            # anchors first (gate the first matmul), warm ACT exp table
            zfa_t = anch.tile([32, 2, A], F8, tag="zfa")
            nc.sync.dma_start(out=zfa_t[:, :, :], in_=zfa_d[:, :, :])
            wu = st.tile([1, 8], F32, tag="wu")
            nc.vector.memset(wu[:, :], 0.0)
            nc.scalar.activation(out=wu[:, :], in_=wu[:, :], func=ACT.Exp, scale=1.0)
            zca_t = anch.tile([64, 2, A], F8, tag="zca")
            nc.sync.dma_start(out=zca_t[:, :, :], in_=zca_d[:, :, :])
            # keys: fg split in 4 pieces so the first groups land early
            zfk_t = keys.tile([32, 2, N], F8, tag="zfk")
            for q in range(4):
                nc.sync.dma_start(out=zfk_t[:, :, q * G:(q + 1) * G],
                                  in_=zfk_d[:, :, q * G:(q + 1) * G])
            zck_t = keys.tile([64, 2, N], F8, tag="zck")
            for q in range(2):
                nc.sync.dma_start(out=zck_t[:, :, q * 2 * G:(q + 1) * 2 * G],
                                  in_=zck_d[:, :, q * 2 * G:(q + 1) * 2 * G])

            # two PSUM regions of [128, 2048] (4 banks each)
            pr = [ps.tile([128, G], F32, tag=f"pr{i}", name=f"pr{i}") for i in range(2)]

            def sims(region, lhsT, keys_t, g, kmax=G):
                for kk in range(int(math.ceil(kmax / 512))):
                    nc.tensor.matmul(
                        out=region[:, kk * 512:(kk + 1) * 512],
                        lhsT=lhsT,
                        rhs=keys_t[:, :, g * G + kk * 512:g * G + (kk + 1) * 512],
                        start=True, stop=True,
                        perf_mode=DR,
                    )

            for ab in range(nblk):
                lf = zfa_t[:, :, ab * 128:(ab + 1) * 128]
                lc = zca_t[:, :, ab * 128:(ab + 1) * 128]
                stt = st.tile([128, NSTAT], F32, tag="stt")

                # --- ACT-owned groups: fg G0,G1 / cls G0,G1 ---
                # fg G0 -> pr0, fg G1 -> pr1; one exp+accum per region, summed
                # into two stat columns (host adds them).  To get a single
                # accum per pair we run one ACT instr per region and chain the
                # second accum via a DVE add?  Simpler: ACT accum per region
                # into separate stat cols -- use cols 0 and 2 halves.
                sims(pr[0], lf, zfk_t, 0)
                sims(pr[1], lf, zfk_t, 1)
                af0 = st.tile([128, 4], F32, tag="af0")
                nc.scalar.activation(out=pr[0][:, :], in_=pr[0][:, :], func=ACT.Exp,
                                     scale=act_scale, accum_out=af0[:, 0:1])
                sims(pr[0], lc, zck_t, 0)
                nc.scalar.activation(out=pr[1][:, :], in_=pr[1][:, :], func=ACT.Exp,
                                     scale=act_scale, accum_out=af0[:, 1:2])
                sims(pr[1], lc, zck_t, 1)
                nc.scalar.activation(out=pr[0][:, :], in_=pr[0][:, :], func=ACT.Exp,
                                     scale=act_scale, accum_out=af0[:, 2:3])
                sims(pr[0], lf, zfk_t, 2)
                nc.scalar.activation(out=pr[1][:, :], in_=pr[1][:, :], func=ACT.Exp,
                                     scale=act_scale, accum_out=af0[:, 3:4])
                # af0 col order is fgG0, fgG1, clsG0, clsG1 (emission order)
                nc.vector.tensor_tensor(out=stt[:, 0:1], in0=af0[:, 0:1],
                                        in1=af0[:, 1:2], op=ALU.add)
                nc.vector.tensor_tensor(out=stt[:, 1:2], in0=af0[:, 2:3],
                                        in1=af0[:, 3:4], op=ALU.add)

                # --- fg G2 (pr0): ACT [0:a2], Pool [a2:] ---
                nc.scalar.activation(out=pr[0][:, 0:a2], in_=pr[0][:, 0:a2],
                                     func=ACT.Exp, scale=act_scale,
                                     accum_out=stt[:, 2:3])
                i16a = sch.tile([128, G], I16, tag="i16a")
                nc.gpsimd.tensor_scalar(out=i16a[:, a2:G], in0=pr[0][:, a2:G],
                                        scalar1=sch_c, scalar2=sch_b,
                                        op0=ALU.mult, op1=ALU.add)
                nc.vector.tensor_reduce(out=stt[:, 3:4],
                                        in_=i16a[:, a2:G].bitcast(BF16),
                                        axis=AX.X, op=ALU.add)
                # --- fg G3 (pr1): Pool all, DVE sums split at bf ---
                sims(pr[1], lf, zfk_t, 3)
                i16b = sch.tile([128, G], I16, tag="i16b")
                nc.gpsimd.tensor_scalar(out=i16b[:, :], in0=pr[1][:, :],
                                        scalar1=sch_c, scalar2=sch_b,
                                        op0=ALU.mult, op1=ALU.add)
                if bf > 0:
                    nc.vector.tensor_reduce(out=stt[:, 4:5],
                                            in_=i16b[:, 0:bf].bitcast(BF16),
                                            axis=AX.X, op=ALU.add)
                else:
                    nc.vector.memset(stt[:, 4:5], 0.0)
                if bf < G:
                    nc.vector.tensor_reduce(out=stt[:, 5:6],
                                            in_=i16b[:, bf:G].bitcast(BF16),
                                            axis=AX.X, op=ALU.add)
                else:
                    nc.vector.memset(stt[:, 5:6], 0.0)
                # --- cls G2 (pr0): Pool ---
                sims(pr[0], lc, zck_t, 2)
                i16c = sch.tile([128, G], I16, tag="i16c")
                nc.gpsimd.tensor_scalar(out=i16c[:, :], in0=pr[0][:, :],
                                        scalar1=sch_c, scalar2=sch_b,
                                        op0=ALU.mult, op1=ALU.add)
                nc.vector.tensor_reduce(out=stt[:, 6:7],
                                        in_=i16c[:, :].bitcast(BF16),
                                        axis=AX.X, op=ALU.add)
                # --- cls G3 (pr1): Pool [0:bc] ---
                sims(pr[1], lc, zck_t, 3, kmax=bc)
                i16d = sch.tile([128, G], I16, tag="i16d")
                if bc > 0:
                    nc.gpsimd.tensor_scalar(out=i16d[:, 0:bc], in0=pr[1][:, 0:bc],
                                            scalar1=sch_c, scalar2=sch_b,
                                            op0=ALU.mult, op1=ALU.add)
                    nc.vector.tensor_reduce(out=stt[:, 7:8],
                                            in_=i16d[:, 0:bc].bitcast(BF16),
                                            axis=AX.X, op=ALU.add)
                else:
                    nc.vector.memset(stt[:, 7:8], 0.0)
                nc.sync.dma_start(out=st_d[ab, :, :], in_=stt[:, :])
    nc.compile()
    return nc


def _run(nc, in_maps, out_names):
    import time as _time

    if os.environ.get("CC_BASS_SIM") == "1":
        from concourse import bass_interp

        ncores = int(os.environ.get("CC_BASS_SIM_CORES", str(N_CORES)))
        results = []
        for m in range(ncores):
            sim = bass_interp.CoreSim(nc, core_id=m)
            for k, v in in_maps[m].items():
                sim.tensor(k)[:] = v
            if nc.partition_id_tensor is not None:
                sim.tensor(nc.partition_id_tensor.name)[:] = np.array(
                    [[m]], dtype=np.uint32
                )
            sim.simulate()
            results.append(
                {name: np.array(sim.mem_tensor(name)) for name in out_names}
            )
        while len(results) < N_CORES:
            results.append(results[-1])
        return results
    t0 = _time.monotonic()
    res = run_bass_kernel_spmd(nc, in_maps, core_ids=list(range(N_CORES)))
    LAST_TIMES.append(_time.monotonic() - t0)
    LAST_RESULTS.append(res)
    return res.results


def kernel(**inputs):
    global LAST_RESULTS, LAST_TIMES
    LAST_RESULTS = []
    LAST_TIMES = []

    roi = np.ascontiguousarray(np.asarray(inputs["roi_feats"], dtype=np.float32))
    labels = np.asarray(inputs["labels"]).astype(np.int64)
    ious = np.asarray(inputs["ious"], dtype=np.float32)
    w1f = np.asarray(inputs["w1f"], dtype=np.float64)
    b1f = np.asarray(inputs["b1f"], dtype=np.float64)
    w2f = np.asarray(inputs["w2f"], dtype=np.float64)
    b2f = np.asarray(inputs["b2f"], dtype=np.float64)
    w1c = np.asarray(inputs["w1c"], dtype=np.float64)
    b1c = np.asarray(inputs["b1c"], dtype=np.float64)
    w2c = np.asarray(inputs["w2c"], dtype=np.float64)
    b2c = np.asarray(inputs["b2c"], dtype=np.float64)
    assert roi.shape == (N, C)

    ign = labels == -1
    fg = (labels > 0) & ~ign
    bg = (labels == 0) & ~ign
    anc = fg & (ious > IOU_THRESHOLD)

    perm = np.concatenate(
        [np.where(anc)[0], np.where(fg & ~anc)[0], np.where(bg)[0], np.where(ign)[0]]
    )
    n_A = int(anc.sum())
    n_fg = int(fg.sum())
    n_valid = n_fg + int(bg.sum())
    if n_A == 0:
        return np.zeros(2, dtype=np.float32)

    x_s = roi[perm]
    labels_s = labels[perm]
    ious_s = ious[perm].astype(np.float64)

    # ---------------- launch 1: MLP ----------------
    # combined layer-1 weights (scaled x64, fp8) for both heads
    w1_all = np.concatenate([w1f, w1c], axis=0)          # [512, 1024]
    b1_all = np.concatenate([b1f, b1c], axis=0) * W1SCALE
    w1_q = (w1_all * W1SCALE).astype(NP8)                # [512, 1024]
    x_q = x_s.astype(NP8)                                # [8192, 1024]

    # layer2 overflow check (fp8 hidden): h = relu(xq @ w1q.T + b1*64)
    h_probe = np.maximum(
        x_q[: 256].astype(np.float32) @ w1_q.astype(np.float32).T
        + b1_all.astype(np.float32), 0)
    layer2_fp8 = bool(h_probe.max() < 200.0)

    mlp_key = ("mlp", layer2_fp8)
    if mlp_key not in _NC_CACHE:
        _NC_CACHE[mlp_key] = _build_mlp_nc(layer2_fp8)
    nc1 = _NC_CACHE[mlp_key]

    KC4 = C // 256
    R = N // N_CORES
    # w1_dr[p, k, i, h] = w1T_all[k*256+i*128+p, h] = w1_all[h, k*256+i*128+p]
    w1_dr = np.ascontiguousarray(
        w1_q.T.reshape(KC4, 2, 128, HID2).transpose(2, 0, 1, 3))
    w2_all = np.concatenate([w2f, w2c], axis=0)          # [192, 256]
    w2dt = NP8 if layer2_fp8 else NPBF
    # w2_dr[p, i, d] = w2_all[d, i*128+p]
    w2_dr = np.ascontiguousarray(
        w2_all.T.reshape(2, 128, DF + DC).transpose(1, 0, 2)).astype(w2dt)
    b1_dr = np.ascontiguousarray(
        b1_all.reshape(HID2 // 128, 128).T).astype(np.float32)

    xT_q = np.ascontiguousarray(x_q.T)                   # [1024, 8192] fp8
    shared1 = {"w1": w1_dr, "w2": w2_dr, "b1": b1_dr}
    in_maps1 = []
    for m in range(N_CORES):
        xm = xT_q[:, m * R:(m + 1) * R]                  # [1024, R]
        x_dr = np.ascontiguousarray(
            xm.reshape(KC4, 2, 128, R).transpose(2, 0, 1, 3))
        in_maps1.append({"x": x_dr, **shared1})
    res1 = _run(nc1, in_maps1, ["zf", "zc"])

    zfT_raw = np.concatenate(
        [r["zf"].astype(np.float64) for r in res1], axis=1)  # [DF, N]
    zcT_raw = np.concatenate(
        [r["zc"].astype(np.float64) for r in res1], axis=1)  # [DC, N]

    # ---------------- host: add b2, normalize in float64 ----------------
    def _normalize(zT_raw, b2):
        z = zT_raw.T + b2[None, :] * W1SCALE
        nrm = np.sqrt(np.sum(z * z, axis=1, keepdims=True)) / W1SCALE
        return z / W1SCALE / np.maximum(nrm, EPS)

    zfn = _normalize(zfT_raw, b2f)                        # [N, DF] fp64, unit rows
    zcn = _normalize(zcT_raw, b2c)                        # [N, DC]

    # fp8 quantized keys (scaled x16) -- exactly what the device will see
    zfq = (zfn * ZSCALE).astype(NP8)
    zcq = (zcn * ZSCALE).astype(NP8)
    zfq64 = zfq.astype(np.float64)
    zcq64 = zcq.astype(np.float64)

    # ---------------- launch 2: sims ----------------
    nblk = max(1, math.ceil(math.ceil(n_A / N_CORES) / 128))
    A_pc = nblk * 128
    # ACT/Pool balance: ACT gets 2*G per head (G0,G1) plus a2 cols of fg G2
    r_cl = n_valid - 3 * G
    a2 = int((1.39 * (2 * G + G + r_cl) - 0.833 * 4 * G - 1650) / 2.223)
    a2 = int(np.clip(a2, 256, G - 256) // 16 * 16)
    sim_key = ("sim", n_fg, n_valid, nblk, a2)
    if sim_key not in _NC_CACHE:
        _NC_CACHE[sim_key] = _build_sim_nc(n_fg, n_valid, nblk, a2)
    nc2 = _NC_CACHE[sim_key]

    # key tensors in DoubleRow layout [d/2, 2, N]
    def _dr_keys(zq, d2):
        # [d, N] -> [d2, 2, N] with slot i = dims [i*d2, (i+1)*d2)
        zT = np.ascontiguousarray(zq.T)                  # [d, N] fp8
        return np.ascontiguousarray(zT.reshape(2, d2, -1).transpose(1, 0, 2))

    zfkT = _dr_keys(zfq, 32)
    zckT = _dr_keys(zcq, 64)
    shared2 = {"zfk": zfkT, "zck": zckT}
    in_maps2 = []
    idx_all = []
    for m in range(N_CORES):
        idx = np.minimum(np.arange(m * A_pc, (m + 1) * A_pc), n_A - 1)
        idx_all.append(idx)
        in_maps2.append({
            "zfa": np.ascontiguousarray(zfkT[:, :, idx]),
            "zca": np.ascontiguousarray(zckT[:, :, idx]),
            **shared2,
        })
    res2 = _run(nc2, in_maps2, ["stats"])

    stats = np.concatenate(
        [r["stats"].reshape(A_pc, 8) for r in res2], axis=0).astype(np.float64)
    stats = stats[np.arange(N_CORES * A_pc) < n_A]        # [n_A, 8]

    # ---------------- host: final losses in float64 ----------------
    act_scale = 1.0 / (ZSCALE * ZSCALE * TAU)
    w_a = ious_s[:n_A]

    # self terms: anchor i's self column is global idx i < n_A <= 2*G,
    # always inside ACT's fg/cls G0+G1 ranges -> exact exp.
    sdot_f = np.einsum("nd,nd->n", zfq64[:n_A], zfq64[:n_A])
    sdot_c = np.einsum("nd,nd->n", zcq64[:n_A], zcq64[:n_A])
    selfexp_f = np.exp(sdot_f * act_scale)
    selfexp_c = np.exp(sdot_c * act_scale)

    numer = stats[:, 0] + stats[:, 2] + stats[:, 3] + stats[:, 4] - selfexp_f
    denom = numer + stats[:, 5]
    denom_c = stats[:, 1] + stats[:, 6] + stats[:, 7] - selfexp_c

    # fg/bg loss
    if n_fg - 1 > 0:
        li = -np.log((numer + EPS) / (denom + EPS))
        loss_fg = np.sum(li * w_a) / (np.sum(w_a) + EPS)
    else:
        loss_fg = 0.0

    # class supcon loss
    lab_valid = labels_s[:n_valid]
    cnt = np.bincount(lab_valid, minlength=21)
    S = np.zeros((21, DC), dtype=np.float64)
    np.add.at(S, lab_valid, zcn[:n_valid])
    c_a = labels_s[:n_A]
    n_pos = (cnt[c_a] - 1).astype(np.float64)
    denom_log = np.log(np.maximum(denom_c, 1e-300))
    zca64 = zcn[:n_A]
    sum_pos = (np.einsum("nd,nd->n", zca64, S[c_a])
               - np.einsum("nd,nd->n", zca64, zca64)) / TAU
    li_c = -(sum_pos - n_pos * denom_log) / np.maximum(n_pos, 1.0)
    valid_c = n_pos > 0
    num2 = np.sum(np.where(valid_c, li_c * w_a, 0.0))
    den2 = np.sum(np.where(valid_c, w_a, 0.0))
    loss_cls = num2 / (den2 + EPS12)

    return np.stack([loss_fg, loss_cls]).astype(np.float32)


# revision 7
# speedup vs baseline: 1.2814x; 1.0961x over previous
"""Trainium2 Bass kernel for nn_MultiHeadContrastive (two-head contrastive loss).

Strategy (8 NeuronCores, two SPMD launches, no collectives):

  Launch 1 (MLP): rows of roi_feats are sorted by group
  (anchor / fg-low-iou / bg / ignore) on the host and sharded contiguously,
  1024 rows per core.  Both layers run in fp8e4(e4m3) with DoubleRow perf
  mode (weights pre-scaled by 64 on the host so fp8 stays in its normal
  range; the scale cancels in the host-side normalization).  Raw
  (pre-normalization, pre-b2) embeddings return as bf16; the host adds b2
  and L2-normalizes in float64.

  Launch 2 (SIM): each core owns nblk*128 anchor rows (lhsT, fp8 x16) and
  all N keys (rhs, fp8-DR).  Keys are RE-ORDERED per core so the core's own
  anchors come first: every anchor's self-similarity column then lands in
  group 0, which is always evaluated by ACT (exact exp), so the host can
  subtract exact self terms.  Per anchor block, sim rows are built in PSUM
  2048 keys at a time (2 regions, double buffered).  exp(sim/TAU) plus the
  masked range sums are computed by THREE engines concurrently:
    - ACT: exact exp + accum_out on whole groups (plus one extension piece)
    - Pool: Schraudolph exp approximation (affine in the exponent, stored
      int16, re-read as bf16) on its column span of the remaining groups
    - DVE: same Schraudolph on its span, plus ALL range sums of the int16
      tiles via tensor_scalar+accum_out (4x DVE perf mode).
  Because rows are sorted, all masked sums are plain column-range sums.

  Host: subtracts exact self terms, computes the class-positive term of the
  SupCon loss from per-class sums of z, applies logs/weights in float64.
"""

import math
import os

import numpy as np
import ml_dtypes

import concourse.bacc as bacc
import concourse.mybir as mybir
import concourse.tile as tile
from concourse.bass_utils import run_bass_kernel_spmd

N_CORES = 8
N, C = 8192, 1024
HID, DF, DC = 256, 64, 128
HID2 = 2 * HID
TAU = 0.2
EPS = 1e-8
EPS12 = 1e-12
IOU_THRESHOLD = 0.5

F32 = mybir.dt.float32
BF16 = mybir.dt.bfloat16
F8 = mybir.dt.float8e4
I16 = mybir.dt.int16
ACT = mybir.ActivationFunctionType
AX = mybir.AxisListType
ALU = mybir.AluOpType
DR = mybir.MatmulPerfMode.DoubleRow

NP8 = ml_dtypes.float8_e4m3
NPBF = ml_dtypes.bfloat16

W1SCALE = 64.0
ZSCALE = 16.0
SCH_SIG = 0.05394  # zero-mean rel. error under float->int16 truncation

LAST_RESULTS = []
LAST_TIMES = []
_NC_CACHE = {}

G = 2048
POOL_FRAC = 0.585  # Pool share of each Pool/DVE group's span


def _schraudolph_consts(psum_scale):
    c = 128.0 * psum_scale * np.log2(np.e)
    b = 128.0 * (127.0 - SCH_SIG)
    return float(c), float(b)


# --------------------------------------------------------------------------
# SIM piece table (shared between device builder and host reassembly)
# --------------------------------------------------------------------------
def _sim_piece_table(n_fg, n_valid, act_ext):
    """Returns (pieces, order).

    pieces: list of dicts: head ('f'|'c'), group g, span [c0,c1) local to the
    group, engine in {'A','P','D'}, plus 'splits': sorted class-boundary cuts
    within the span (absolute head-column space).  Each (engine-span x split
    interval) becomes one stat column, assigned in order.

    order: group emission order [(head, g), ...] interleaving ACT-owned and
    Pool/DVE-owned groups for engine overlap.
    """
    bf = n_fg - 3 * G            # fg class boundary inside fg G3
    bc = n_valid - 3 * G         # cls valid limit inside cls G3
    assert 0 < bf <= G and 0 < bc <= G

    # group -> owner: ACT-pure: fg G0, cls G0, fg G1. Mixed: cls G1
    # ([0:act_ext) ACT, rest Pool/DVE). Pool/DVE: fg G2, fg G3, cls G2, cls G3.
    order = [("f", 0), ("f", 2), ("c", 0), ("f", 3), ("f", 1), ("c", 2),
             ("c", 1), ("c", 3)]
    pieces = []

    def add(head, g, c0, c1, eng, cuts=()):
        lo = g * G
        cs = sorted({c0 + lo, c1 + lo} | {c for c in cuts if c0 + lo < c < c1 + lo})
        for a, b in zip(cs[:-1], cs[1:]):
            pieces.append(dict(head=head, g=g, c0=a - lo, c1=b - lo, eng=eng))

    def pd(head, g, c0, c1, cuts=()):
        cp = c0 + int(round(POOL_FRAC * (c1 - c0)))
        add(head, g, c0, cp, "P", cuts)
        add(head, g, cp, c1, "D", cuts)

    add("f", 0, 0, G, "A")
    add("f", 1, 0, G, "A")
    add("c", 0, 0, G, "A")
    add("c", 1, 0, act_ext, "A")
    pd("c", 1, act_ext, G)
    pd("f", 2, 0, G)
    pd("f", 3, 0, G, cuts=(n_fg,))
    pd("c", 2, 0, G)
    pd("c", 3, 0, bc)
    for i, p in enumerate(pieces):
        p["stat"] = i
    return pieces, order


# --------------------------------------------------------------------------
# Launch 1: MLP (both heads, fp8 DoubleRow)
# --------------------------------------------------------------------------
def _build_mlp_nc(layer2_fp8):
    R = N // N_CORES
    RB = 512
    NR = R // RB
    KC4 = C // 256
    NH4 = HID2 // 128

    nc = bacc.Bacc(trn_type="TRN2", num_devices=N_CORES, debug=False)
    x_d = nc.dram_tensor("x", [128, KC4, 2, R], F8, kind="ExternalInput")
    w1_d = nc.dram_tensor("w1", [128, KC4, 2, HID2], F8, kind="ExternalInput")
    w2dt = F8 if layer2_fp8 else BF16
    w2_d = nc.dram_tensor("w2", [128, 2, DF + DC], w2dt, kind="ExternalInput")
    b1_d = nc.dram_tensor("b1", [128, NH4], F32, kind="ExternalInput")
    zf_d = nc.dram_tensor("zf", [DF, R], BF16, kind="ExternalOutput")
    zc_d = nc.dram_tensor("zc", [DC, R], BF16, kind="ExternalOutput")

    hdt = F8 if layer2_fp8 else BF16
    with tile.TileContext(nc) as tc:
        with (
            tc.tile_pool(name="cst", bufs=1) as cst,
            tc.tile_pool(name="hb", bufs=2) as hb,
            tc.tile_pool(name="zb", bufs=2) as zb,
            tc.tile_pool(name="ps", bufs=1, space="PSUM") as ps,
        ):
            wu = cst.tile([1, 8], F32, tag="wu")
            nc.vector.memset(wu[:, :], 0.0)
            nc.scalar.activation(out=wu[:, :], in_=wu[:, :], func=ACT.Relu, scale=1.0)

            # DMA order: w1(k01), x0(k01), w1(k23), x0(k23), x1, b1, w2
            w1_t = cst.tile([128, KC4, 2, HID2], F8, tag="w1")
            x_t = cst.tile([128, KC4, 2, R], F8, tag="x")
            nc.sync.dma_start(out=w1_t[:, 0:2, :, :], in_=w1_d[:, 0:2, :, :])
            nc.sync.dma_start(out=x_t[:, 0:2, :, 0:RB], in_=x_d[:, 0:2, :, 0:RB])
            nc.sync.dma_start(out=w1_t[:, 2:4, :, :], in_=w1_d[:, 2:4, :, :])
            nc.sync.dma_start(out=x_t[:, 2:4, :, 0:RB], in_=x_d[:, 2:4, :, 0:RB])
            nc.sync.dma_start(out=x_t[:, :, :, RB:R], in_=x_d[:, :, :, RB:R])
            b1_t = cst.tile([128, NH4], F32, tag="b1")
            nc.sync.dma_start(out=b1_t[:, :], in_=b1_d[:, :])
            w2_t = cst.tile([128, 2, DF + DC], w2dt, tag="w2")
            nc.sync.dma_start(out=w2_t[:, :, :], in_=w2_d[:, :, :])

            # 8 psum banks: hp[r][h4] for both rblocks; z reuses drained banks
            hp = {(r, h4): ps.tile([128, RB], F32, tag=f"hp{r}{h4}",
                                   name=f"hp{r}{h4}")
                  for r in range(NR) for h4 in range(NH4)}
            for r in range(NR):
                hsb = hb.tile([128, NH4, RB], hdt, tag=f"h{r}")
                for h4 in range(NH4):
                    for k in range(KC4):
                        nc.tensor.matmul(
                            out=hp[(r, h4)][:, :],
                            lhsT=w1_t[:, k, :, h4 * 128:(h4 + 1) * 128],
                            rhs=x_t[:, k, :, r * RB:(r + 1) * RB],
                            start=(k == 0),
                            stop=(k == KC4 - 1),
                            perf_mode=DR,
                        )
                    if h4 % 2 == 0:
                        nc.vector.tensor_scalar(
                            out=hsb[:, h4, :], in0=hp[(r, h4)][:, :],
                            scalar1=b1_t[:, h4:h4 + 1], scalar2=0.0,
                            op0=ALU.add, op1=ALU.max)
                    else:
                        nc.scalar.activation(
                            out=hsb[:, h4, :], in_=hp[(r, h4)][:, :],
                            func=ACT.Relu, bias=b1_t[:, h4:h4 + 1], scale=1.0)
                for hi, (d, zd) in enumerate(((DF, zf_d), (DC, zc_d))):
                    c0 = 0 if hi == 0 else DF
                    zp = ps.tile([d, RB], F32, tag=f"hp{r}{hi}", name=f"zp{r}{hi}")
                    if layer2_fp8:
                        nc.tensor.matmul(
                            out=zp[:, :], lhsT=w2_t[:, :, c0:c0 + d],
                            rhs=hsb[:, 2 * hi:2 * hi + 2, :],
                            start=True, stop=True, perf_mode=DR)
                    else:
                        for kk in range(2):
                            nc.tensor.matmul(
                                out=zp[:, :], lhsT=w2_t[:, kk, c0:c0 + d],
                                rhs=hsb[:, 2 * hi + kk, :],
                                start=(kk == 0), stop=(kk == 1))
                    zt = zb.tile([d, RB], BF16, tag=f"z{hi}")
                    if hi == 0:
                        nc.vector.tensor_scalar(
                            out=zt[:, :], in0=zp[:, :], scalar1=1.0,
                            scalar2=None, op0=ALU.mult)
                    else:
                        nc.scalar.activation(
                            out=zt[:, :], in_=zp[:, :], func=ACT.Copy,
                            bias=0.0, scale=1.0)
                    nc.sync.dma_start(out=zd[:, r * RB:(r + 1) * RB], in_=zt[:, :])
    nc.compile()
    return nc


# --------------------------------------------------------------------------
# Launch 2: SIM
# --------------------------------------------------------------------------
def _build_sim_nc(n_fg, n_valid, nblk, act_ext):
    A = nblk * 128
    pieces, order = _sim_piece_table(n_fg, n_valid, act_ext)
    NSTAT = len(pieces)
    act_scale = 1.0 / (ZSCALE * ZSCALE * TAU)
    sch_c, sch_b = _schraudolph_consts(act_scale)

    nc = bacc.Bacc(trn_type="TRN2", num_devices=N_CORES, debug=False)
    zfk_d = nc.dram_tensor("zfk", [32, 2, N], F8, kind="ExternalInput")
    zck_d = nc.dram_tensor("zck", [64, 2, N], F8, kind="ExternalInput")
    zfa_d = nc.dram_tensor("zfa", [32, 2, A], F8, kind="ExternalInput")
    zca_d = nc.dram_tensor("zca", [64, 2, A], F8, kind="ExternalInput")
    st_d = nc.dram_tensor("stats", [nblk, 128, NSTAT], F32, kind="ExternalOutput")

    # per (head, group): list of pieces
    by_group = {}
    for p in pieces:
        by_group.setdefault((p["head"], p["g"]), []).append(p)

    with tile.TileContext(nc) as tc:
        with (
            tc.tile_pool(name="keys", bufs=1) as keys,
            tc.tile_pool(name="anch", bufs=1) as anch,
            tc.tile_pool(name="sch", bufs=2) as sch,
            tc.tile_pool(name="st", bufs=2) as st,
            tc.tile_pool(name="ps", bufs=1, space="PSUM") as ps,
        ):
            zfa_t = anch.tile([32, 2, A], F8, tag="zfa")
            nc.sync.dma_start(out=zfa_t[:, :, :], in_=zfa_d[:, :, :])
            wu = st.tile([1, 8], F32, tag="wu")
            nc.vector.memset(wu[:, :], 0.0)
            nc.scalar.activation(out=wu[:, :], in_=wu[:, :], func=ACT.Exp, scale=1.0)
            zfk_t = keys.tile([32, 2, N], F8, tag="zfk")
            nc.sync.dma_start(out=zfk_t[:, :, 0:N // 2], in_=zfk_d[:, :, 0:N // 2])
            zca_t = anch.tile([64, 2, A], F8, tag="zca")
            nc.sync.dma_start(out=zca_t[:, :, :], in_=zca_d[:, :, :])
            zck_t = keys.tile([64, 2, N], F8, tag="zck")
            nc.sync.dma_start(out=zck_t[:, :, 0:N // 2], in_=zck_d[:, :, 0:N // 2])
            nc.sync.dma_start(out=zfk_t[:, :, N // 2:N], in_=zfk_d[:, :, N // 2:N])
            nc.sync.dma_start(out=zck_t[:, :, N // 2:N], in_=zck_d[:, :, N // 2:N])

            pr = [ps.tile([128, G], F32, tag=f"pr{i}", name=f"pr{i}")
                  for i in range(2)]

            for ab in range(nblk):
                stt = st.tile([128, NSTAT], F32, tag="stt")
                dummy = st.tile([128, G], BF16, tag="dummy")
                for oi, (head, g) in enumerate(order):
                    region = pr[oi % 2]
                    lhsT = (zfa_t if head == "f" else zca_t)[:, :, ab * 128:(ab + 1) * 128]
                    keys_t = zfk_t if head == "f" else zck_t
                    gp = by_group[(head, g)]
                    kmax = max(p["c1"] for p in gp)
                    for kk in range(int(math.ceil(kmax / 512))):
                        nc.tensor.matmul(
                            out=region[:, kk * 512:(kk + 1) * 512],
                            lhsT=lhsT,
                            rhs=keys_t[:, :, g * G + kk * 512:g * G + (kk + 1) * 512],
                            start=True, stop=True, perf_mode=DR)
                    # consumers: ACT pieces first, then Pool/DVE mains, then sums
                    i16 = None
                    if any(p["eng"] in "PD" for p in gp):
                        i16 = sch.tile([128, G], I16, tag=f"i16{oi % 4}")
                    for p in gp:
                        if p["eng"] == "A":
                            nc.scalar.activation(
                                out=region[:, p["c0"]:p["c1"]],
                                in_=region[:, p["c0"]:p["c1"]],
                                func=ACT.Exp, scale=act_scale,
                                accum_out=stt[:, p["stat"]:p["stat"] + 1])
                    # Pool span, then DVE span (each may be split by class cuts;
                    # emit one engine op per contiguous engine span)
                    for eng, engine in (("P", nc.gpsimd), ("D", nc.vector)):
                        sp = [p for p in gp if p["eng"] == eng]
                        if not sp:
                            continue
                        c0 = min(p["c0"] for p in sp)
                        c1 = max(p["c1"] for p in sp)
                        engine.tensor_scalar(
                            out=i16[:, c0:c1], in0=region[:, c0:c1],
                            scalar1=sch_c, scalar2=sch_b,
                            op0=ALU.mult, op1=ALU.add)
                    for p in gp:
                        if p["eng"] in "PD":
                            nc.vector.tensor_scalar(
                                out=dummy[:, p["c0"]:p["c1"]],
                                in0=i16[:, p["c0"]:p["c1"]].bitcast(BF16),
                                scalar1=1.0, scalar2=0.0,
                                op0=ALU.mult, op1=ALU.add,
                                accum_out=stt[:, p["stat"]:p["stat"] + 1])
                nc.sync.dma_start(out=st_d[ab, :, :], in_=stt[:, :])
    nc.compile()
    return nc


def _run(nc, in_maps, out_names):
    import time as _time

    if os.environ.get("CC_BASS_SIM") == "1":
        from concourse import bass_interp

        ncores = int(os.environ.get("CC_BASS_SIM_CORES", str(N_CORES)))
        results = []
        for m in range(ncores):
            sim = bass_interp.CoreSim(nc, core_id=m)
            for k, v in in_maps[m].items():
                sim.tensor(k)[:] = v
            if nc.partition_id_tensor is not None:
                sim.tensor(nc.partition_id_tensor.name)[:] = np.array(
                    [[m]], dtype=np.uint32)
            sim.simulate()
            results.append(
                {name: np.array(sim.mem_tensor(name)) for name in out_names})
        while len(results) < N_CORES:
            results.append(results[-1])
        return results
    t0 = _time.monotonic()
    res = run_bass_kernel_spmd(nc, in_maps, core_ids=list(range(N_CORES)))
    LAST_TIMES.append(_time.monotonic() - t0)
    LAST_RESULTS.append(res)
    return res.results


def kernel(**inputs):
    global LAST_RESULTS, LAST_TIMES
    LAST_RESULTS = []
    LAST_TIMES = []

    roi = np.ascontiguousarray(np.asarray(inputs["roi_feats"], dtype=np.float32))
    labels = np.asarray(inputs["labels"]).astype(np.int64)
    ious = np.asarray(inputs["ious"], dtype=np.float32)
    w1f = np.asarray(inputs["w1f"], dtype=np.float64)
    b1f = np.asarray(inputs["b1f"], dtype=np.float64)
    w2f = np.asarray(inputs["w2f"], dtype=np.float64)
    b2f = np.asarray(inputs["b2f"], dtype=np.float64)
    w1c = np.asarray(inputs["w1c"], dtype=np.float64)
    b1c = np.asarray(inputs["b1c"], dtype=np.float64)
    w2c = np.asarray(inputs["w2c"], dtype=np.float64)
    b2c = np.asarray(inputs["b2c"], dtype=np.float64)
    assert roi.shape == (N, C)

    ign = labels == -1
    fg = (labels > 0) & ~ign
    bg = (labels == 0) & ~ign
    anc = fg & (ious > IOU_THRESHOLD)

    perm = np.concatenate(
        [np.where(anc)[0], np.where(fg & ~anc)[0], np.where(bg)[0], np.where(ign)[0]])
    n_A = int(anc.sum())
    n_fg = int(fg.sum())
    n_valid = n_fg + int(bg.sum())
    if n_A == 0:
        return np.zeros(2, dtype=np.float32)

    x_s = roi[perm]
    labels_s = labels[perm]
    ious_s = ious[perm].astype(np.float64)

    # ---------------- launch 1: MLP ----------------
    w1_all = np.concatenate([w1f, w1c], axis=0)
    b1_all = np.concatenate([b1f, b1c], axis=0) * W1SCALE
    w1_q = (w1_all * W1SCALE).astype(NP8)
    x_q = x_s.astype(NP8)

    h_probe = np.maximum(
        x_q[:256].astype(np.float32) @ w1_q.astype(np.float32).T
        + b1_all.astype(np.float32), 0)
    layer2_fp8 = bool(h_probe.max() < 200.0)

    mlp_key = ("mlp", layer2_fp8)
    if mlp_key not in _NC_CACHE:
        _NC_CACHE[mlp_key] = _build_mlp_nc(layer2_fp8)
    nc1 = _NC_CACHE[mlp_key]

    KC4 = C // 256
    R = N // N_CORES
    w1_dr = np.ascontiguousarray(
        w1_q.T.reshape(KC4, 2, 128, HID2).transpose(2, 0, 1, 3))
    w2_all = np.concatenate([w2f, w2c], axis=0)
    w2dt = NP8 if layer2_fp8 else NPBF
    w2_dr = np.ascontiguousarray(
        w2_all.T.reshape(2, 128, DF + DC).transpose(1, 0, 2)).astype(w2dt)
    b1_dr = np.ascontiguousarray(
        b1_all.reshape(HID2 // 128, 128).T).astype(np.float32)

    xT_q = np.ascontiguousarray(x_q.T)
    shared1 = {"w1": w1_dr, "w2": w2_dr, "b1": b1_dr}
    in_maps1 = []
    for m in range(N_CORES):
        xm = xT_q[:, m * R:(m + 1) * R]
        x_dr = np.ascontiguousarray(
            xm.reshape(KC4, 2, 128, R).transpose(2, 0, 1, 3))
        in_maps1.append({"x": x_dr, **shared1})
    res1 = _run(nc1, in_maps1, ["zf", "zc"])

    zfT_raw = np.concatenate([r["zf"].astype(np.float64) for r in res1], axis=1)
    zcT_raw = np.concatenate([r["zc"].astype(np.float64) for r in res1], axis=1)

    def _normalize(zT_raw, b2):
        z = zT_raw.T + b2[None, :] * W1SCALE
        nrm = np.sqrt(np.sum(z * z, axis=1, keepdims=True)) / W1SCALE
        return z / W1SCALE / np.maximum(nrm, EPS)

    zfn = _normalize(zfT_raw, b2f)
    zcn = _normalize(zcT_raw, b2c)

    zfq = (zfn * ZSCALE).astype(NP8)
    zcq = (zcn * ZSCALE).astype(NP8)
    zfq64 = zfq.astype(np.float64)
    zcq64 = zcq.astype(np.float64)

    # ---------------- launch 2: sims ----------------
    nblk = max(1, math.ceil(math.ceil(n_A / N_CORES) / 128))
    A_pc = nblk * 128
    # ACT extension into cls G1 to balance engines (rounded to 16)
    act_ext = 966 // 16 * 16
    sim_key = ("sim", n_fg, n_valid, nblk, act_ext)
    if sim_key not in _NC_CACHE:
        _NC_CACHE[sim_key] = _build_sim_nc(n_fg, n_valid, nblk, act_ext)
    nc2 = _NC_CACHE[sim_key]
    pieces, _ = _sim_piece_table(n_fg, n_valid, act_ext)

    def _dr(zq_cols):
        d = zq_cols.shape[0]
        return np.ascontiguousarray(zq_cols.reshape(2, d // 2, -1).transpose(1, 0, 2))

    zfqT = np.ascontiguousarray(zfq.T)   # [DF, N]
    zcqT = np.ascontiguousarray(zcq.T)   # [DC, N]
    in_maps2 = []
    for m in range(N_CORES):
        lo = min(m * A_pc, n_A)
        hi = min((m + 1) * A_pc, n_A)
        # local key order: own anchor window first
        local = np.concatenate([
            np.arange(lo, hi),
            np.arange(0, lo),
            np.arange(hi, N),
        ])
        aidx = np.minimum(np.arange(m * A_pc, (m + 1) * A_pc), n_A - 1)
        in_maps2.append({
            "zfk": _dr(zfqT[:, local]),
            "zck": _dr(zcqT[:, local]),
            "zfa": _dr(zfqT[:, aidx]),
            "zca": _dr(zcqT[:, aidx]),
        })
    res2 = _run(nc2, in_maps2, ["stats"])

    NSTAT = len(pieces)
    stats = np.stack([r["stats"].reshape(A_pc, NSTAT) for r in res2], axis=0)
    stats = stats.astype(np.float64)      # [cores, A_pc, NSTAT]

    # ---------------- host: final losses in float64 ----------------
    act_scale = 1.0 / (ZSCALE * ZSCALE * TAU)
    # piece -> class membership (per-core local column space; class sections
    # are preserved by the local reordering, so boundaries are global)
    numer_cols = [p["stat"] for p in pieces
                  if p["head"] == "f" and p["g"] * G + p["c1"] <= n_fg]
    denom_cols = [p["stat"] for p in pieces if p["head"] == "f"]
    denc_cols = [p["stat"] for p in pieces if p["head"] == "c"]

    out_rows = np.empty((n_A, NSTAT), dtype=np.float64)
    for m in range(N_CORES):
        lo = m * A_pc
        hi = min((m + 1) * A_pc, n_A)
        if hi > lo:
            out_rows[lo:hi] = stats[m, : hi - lo]
    stats = out_rows                      # [n_A, NSTAT]

    w_a = ious_s[:n_A]
    sdot_f = np.einsum("nd,nd->n", zfq64[:n_A], zfq64[:n_A])
    sdot_c = np.einsum("nd,nd->n", zcq64[:n_A], zcq64[:n_A])
    selfexp_f = np.exp(sdot_f * act_scale)
    selfexp_c = np.exp(sdot_c * act_scale)

    numer = stats[:, numer_cols].sum(1) - selfexp_f
    denom = stats[:, denom_cols].sum(1) - selfexp_f
    denom_c = stats[:, denc_cols].sum(1) - selfexp_c

    if n_fg - 1 > 0:
        li = -np.log((numer + EPS) / (denom + EPS))
        loss_fg = np.sum(li * w_a) / (np.sum(w_a) + EPS)
    else:
        loss_fg = 0.0

    lab_valid = labels_s[:n_valid]
    cnt = np.bincount(lab_valid, minlength=21)
    S = np.zeros((21, DC), dtype=np.float64)
    np.add.at(S, lab_valid, zcn[:n_valid])
    c_a = labels_s[:n_A]
    n_pos = (cnt[c_a] - 1).astype(np.float64)
    denom_log = np.log(np.maximum(denom_c, 1e-300))
    zca64 = zcn[:n_A]
    sum_pos = (np.einsum("nd,nd->n", zca64, S[c_a])
               - np.einsum("nd,nd->n", zca64, zca64)) / TAU
    li_c = -(sum_pos - n_pos * denom_log) / np.maximum(n_pos, 1.0)
    valid_c = n_pos > 0
    num2 = np.sum(np.where(valid_c, li_c * w_a, 0.0))
    den2 = np.sum(np.where(valid_c, w_a, 0.0))
    loss_cls = num2 / (den2 + EPS12)

    return np.stack([loss_fg, loss_cls]).astype(np.float32)


# revision 11
# speedup vs baseline: 1.5660x; 1.2220x over previous
"""Trainium2 Bass kernel for nn_MultiHeadContrastive (two-head contrastive loss).

Strategy (8 NeuronCores, two SPMD launches, no collectives):

  Launch 1 (MLP): rows of roi_feats are sorted by group
  (anchor / fg-low-iou / bg / ignore) on the host and sharded contiguously,
  1024 rows per core.  Both layers run in fp8e4(e4m3) with DoubleRow perf
  mode (weights pre-scaled by 64 on the host so fp8 stays in its normal
  range; the scale cancels in the host-side normalization).  Raw
  (pre-normalization, pre-b2) embeddings return as bf16; the host adds b2
  and L2-normalizes in float64.

  Launch 2 (SIM): each core owns nblk*128 anchor rows (lhsT, fp8 x16) and
  all N keys (rhs, fp8-DR).  Keys are RE-ORDERED per core so the core's own
  anchors come first: every anchor's self-similarity column then lands in
  group 0, which is always evaluated by ACT (exact exp), so the host can
  subtract exact self terms.  Per anchor block, sim rows are built in PSUM
  2048 keys at a time (2 regions, double buffered).  exp(sim/TAU) plus the
  masked range sums are computed by THREE engines concurrently:
    - ACT: exact exp + accum_out on whole groups (plus one extension piece)
    - Pool: Schraudolph exp approximation (affine in the exponent, stored
      int16, re-read as bf16) on its column span of the remaining groups
    - DVE: same Schraudolph on its span, plus ALL range sums of the int16
      tiles via tensor_scalar+accum_out (4x DVE perf mode).
  Because rows are sorted, all masked sums are plain column-range sums.

  Host: subtracts exact self terms, computes the class-positive term of the
  SupCon loss from per-class sums of z, applies logs/weights in float64.
"""

import math
import os

import numpy as np
import ml_dtypes

import concourse.bacc as bacc
import concourse.mybir as mybir
import concourse.tile as tile
from concourse.bass_utils import run_bass_kernel_spmd

N_CORES = 8
N, C = 8192, 1024
HID, DF, DC = 256, 64, 128
HID2 = 2 * HID
TAU = 0.2
EPS = 1e-8
EPS12 = 1e-12
IOU_THRESHOLD = 0.5

F32 = mybir.dt.float32
BF16 = mybir.dt.bfloat16
F8 = mybir.dt.float8e4
I16 = mybir.dt.int16
ACT = mybir.ActivationFunctionType
AX = mybir.AxisListType
ALU = mybir.AluOpType
DR = mybir.MatmulPerfMode.DoubleRow

NP8 = ml_dtypes.float8_e4m3
NPBF = ml_dtypes.bfloat16

W1SCALE = 64.0
ZSCALE = 16.0
SCH_SIG = 0.05394  # zero-mean rel. error under float->int16 truncation

LAST_RESULTS = []
LAST_TIMES = []
_NC_CACHE = {}

G = 1024
POOL_FRAC = 0.62   # Pool share of each Pool/DVE group's span


def _schraudolph_consts(psum_scale):
    c = 128.0 * psum_scale * np.log2(np.e)
    b = 128.0 * (127.0 - SCH_SIG)
    return float(c), float(b)


# --------------------------------------------------------------------------
# SIM piece table (shared between device builder and host reassembly)
# --------------------------------------------------------------------------
def _sim_piece_table(n_fg, n_valid, act_ext):
    """Returns (pieces, order).

    pieces: list of dicts: head ('f'|'c'), group g, span [c0,c1) local to the
    group, engine in {'A','P','D'}, plus 'splits': sorted class-boundary cuts
    within the span (absolute head-column space).  Each (engine-span x split
    interval) becomes one stat column, assigned in order.

    order: group emission order [(head, g), ...] interleaving ACT-owned and
    Pool/DVE-owned groups for engine overlap.
    """
    NGH = N // G                  # 8 groups per head
    gf_cut = n_fg // G            # fg class boundary group
    bc_g = n_valid // G           # last (partial) cls group
    bc = n_valid - bc_g * G
    assert n_fg < N and 0 < bc <= G

    # ACT owns fg g0..g3, cls g0..g1 fully, plus cls g2[0:act_ext); selves
    # (global col < nblk*128 <= 1024) are inside fg g0 / cls g0.
    pieces = []

    def add(head, g, c0, c1, eng, cuts=()):
        lo = g * G
        cs = sorted({c0 + lo, c1 + lo} | {c for c in cuts if c0 + lo < c < c1 + lo})
        for a, b in zip(cs[:-1], cs[1:]):
            pieces.append(dict(head=head, g=g, c0=a - lo, c1=b - lo, eng=eng))

    def pd(head, g, c0, c1, cuts=()):
        cp = c0 + int(round(POOL_FRAC * (c1 - c0)))
        add(head, g, c0, cp, "P", cuts)
        if cp < c1:
            add(head, g, cp, c1, "D", cuts)

    for g in range(4):
        add("f", g, 0, G, "A")
    for g in range(4, NGH):
        pd("f", g, 0, G, cuts=(n_fg,))
    add("c", 0, 0, G, "A")
    add("c", 1, 0, G, "A")
    add("c", 2, 0, act_ext, "A")
    pd("c", 2, act_ext, G)
    for g in range(3, bc_g):
        pd("c", g, 0, G)
    if bc > 0:
        pd("c", bc_g, 0, bc)
    for i, p in enumerate(pieces):
        p["stat"] = i

    # emission order: interleave ACT groups (even slots) with Pool/DVE groups
    a_groups = [("f", 0), ("c", 0), ("f", 1), ("c", 1), ("f", 2), ("f", 3),
                ("c", 2)]
    pd_groups = [("f", g) for g in range(4, NGH)]
    pd_groups += [("c", g) for g in range(3, bc_g + (1 if bc > 0 else 0))]
    order = []
    ai, pi = 0, 0
    for i in range(len(a_groups) + len(pd_groups)):
        take_a = (i % 2 == 0 and ai < len(a_groups)) or pi >= len(pd_groups)
        if take_a:
            order.append(a_groups[ai]); ai += 1
        else:
            order.append(pd_groups[pi]); pi += 1
    return pieces, order


# --------------------------------------------------------------------------
# Launch 1: MLP (both heads, fp8 DoubleRow)
# --------------------------------------------------------------------------
def _build_mlp_nc(layer2_fp8):
    R = N // N_CORES
    RB = 512
    NR = R // RB
    KC4 = C // 256
    NH4 = HID2 // 128

    nc = bacc.Bacc(trn_type="TRN2", num_devices=N_CORES, debug=False)
    x_d = nc.dram_tensor("x", [128, KC4, 2, R], F8, kind="ExternalInput")
    w1_d = nc.dram_tensor("w1", [128, KC4, 2, HID2], F8, kind="ExternalInput")
    w2dt = F8 if layer2_fp8 else BF16
    w2_d = nc.dram_tensor("w2", [128, 2, DF + DC], w2dt, kind="ExternalInput")
    b1_d = nc.dram_tensor("b1", [128, NH4], F32, kind="ExternalInput")
    zf_d = nc.dram_tensor("zf", [DF, R], BF16, kind="ExternalOutput")
    zc_d = nc.dram_tensor("zc", [DC, R], BF16, kind="ExternalOutput")

    hdt = F8 if layer2_fp8 else BF16
    with tile.TileContext(nc) as tc:
        with (
            tc.tile_pool(name="cst", bufs=1) as cst,
            tc.tile_pool(name="hb", bufs=2) as hb,
            tc.tile_pool(name="zb", bufs=2) as zb,
            tc.tile_pool(name="ps", bufs=1, space="PSUM") as ps,
        ):
            wu = cst.tile([1, 8], F32, tag="wu")
            nc.vector.memset(wu[:, :], 0.0)
            nc.scalar.activation(out=wu[:, :], in_=wu[:, :], func=ACT.Relu, scale=1.0)

            # DMA order: w1(k01), x0(k01), w1(k23), x0(k23), x1, b1, w2
            w1_t = cst.tile([128, KC4, 2, HID2], F8, tag="w1")
            x_t = cst.tile([128, KC4, 2, R], F8, tag="x")
            nc.sync.dma_start(out=w1_t[:, 0:2, :, :], in_=w1_d[:, 0:2, :, :])
            nc.sync.dma_start(out=x_t[:, 0:2, :, 0:RB], in_=x_d[:, 0:2, :, 0:RB])
            nc.sync.dma_start(out=w1_t[:, 2:4, :, :], in_=w1_d[:, 2:4, :, :])
            nc.sync.dma_start(out=x_t[:, 2:4, :, 0:RB], in_=x_d[:, 2:4, :, 0:RB])
            nc.sync.dma_start(out=x_t[:, :, :, RB:R], in_=x_d[:, :, :, RB:R])
            b1_t = cst.tile([128, NH4], F32, tag="b1")
            nc.sync.dma_start(out=b1_t[:, :], in_=b1_d[:, :])
            w2_t = cst.tile([128, 2, DF + DC], w2dt, tag="w2")
            nc.sync.dma_start(out=w2_t[:, :, :], in_=w2_d[:, :, :])

            # 8 psum banks: hp[r][h4] for both rblocks; z reuses drained banks
            hp = {(r, h4): ps.tile([128, RB], F32, tag=f"hp{r}{h4}",
                                   name=f"hp{r}{h4}")
                  for r in range(NR) for h4 in range(NH4)}
            for r in range(NR):
                hsb = hb.tile([128, NH4, RB], hdt, tag=f"h{r}")
                for h4 in range(NH4):
                    for k in range(KC4):
                        nc.tensor.matmul(
                            out=hp[(r, h4)][:, :],
                            lhsT=w1_t[:, k, :, h4 * 128:(h4 + 1) * 128],
                            rhs=x_t[:, k, :, r * RB:(r + 1) * RB],
                            start=(k == 0),
                            stop=(k == KC4 - 1),
                            perf_mode=DR,
                        )
                    if h4 % 2 == 0:
                        nc.vector.tensor_scalar(
                            out=hsb[:, h4, :], in0=hp[(r, h4)][:, :],
                            scalar1=b1_t[:, h4:h4 + 1], scalar2=0.0,
                            op0=ALU.add, op1=ALU.max)
                    else:
                        nc.scalar.activation(
                            out=hsb[:, h4, :], in_=hp[(r, h4)][:, :],
                            func=ACT.Relu, bias=b1_t[:, h4:h4 + 1], scale=1.0)
                for hi, (d, zd) in enumerate(((DF, zf_d), (DC, zc_d))):
                    c0 = 0 if hi == 0 else DF
                    zp = ps.tile([d, RB], F32, tag=f"hp{r}{hi}", name=f"zp{r}{hi}")
                    if layer2_fp8:
                        nc.tensor.matmul(
                            out=zp[:, :], lhsT=w2_t[:, :, c0:c0 + d],
                            rhs=hsb[:, 2 * hi:2 * hi + 2, :],
                            start=True, stop=True, perf_mode=DR)
                    else:
                        for kk in range(2):
                            nc.tensor.matmul(
                                out=zp[:, :], lhsT=w2_t[:, kk, c0:c0 + d],
                                rhs=hsb[:, 2 * hi + kk, :],
                                start=(kk == 0), stop=(kk == 1))
                    zt = zb.tile([d, RB], BF16, tag=f"z{hi}")
                    if hi == 0:
                        nc.vector.tensor_scalar(
                            out=zt[:, :], in0=zp[:, :], scalar1=1.0,
                            scalar2=None, op0=ALU.mult)
                    else:
                        nc.scalar.activation(
                            out=zt[:, :], in_=zp[:, :], func=ACT.Copy,
                            bias=0.0, scale=1.0)
                    nc.sync.dma_start(out=zd[:, r * RB:(r + 1) * RB], in_=zt[:, :])
    nc.compile()
    return nc


# --------------------------------------------------------------------------
# Launch 2: SIM
# --------------------------------------------------------------------------
def _build_sim_nc(n_fg, n_valid, nblk, act_ext):
    A = nblk * 128
    pieces, order = _sim_piece_table(n_fg, n_valid, act_ext)
    NSTAT = len(pieces)
    act_scale = 1.0 / (ZSCALE * ZSCALE * TAU)
    sch_c, sch_b = _schraudolph_consts(act_scale)

    nc = bacc.Bacc(trn_type="TRN2", num_devices=N_CORES, debug=False)
    zfk_d = nc.dram_tensor("zfk", [32, 2, N], F8, kind="ExternalInput")
    zck_d = nc.dram_tensor("zck", [64, 2, N], F8, kind="ExternalInput")
    zfa_d = nc.dram_tensor("zfa", [32, 2, A], F8, kind="ExternalInput")
    zca_d = nc.dram_tensor("zca", [64, 2, A], F8, kind="ExternalInput")
    st_d = nc.dram_tensor("stats", [nblk, 128, NSTAT], F32, kind="ExternalOutput")

    # per (head, group): list of pieces
    by_group = {}
    for p in pieces:
        by_group.setdefault((p["head"], p["g"]), []).append(p)

    with tile.TileContext(nc) as tc:
        with (
            tc.tile_pool(name="keys", bufs=1) as keys,
            tc.tile_pool(name="anch", bufs=1) as anch,
            tc.tile_pool(name="sch", bufs=2) as sch,
            tc.tile_pool(name="st", bufs=2) as st,
            tc.tile_pool(name="ps", bufs=1, space="PSUM") as ps,
        ):
            zfa_t = anch.tile([32, 2, A], F8, tag="zfa")
            nc.sync.dma_start(out=zfa_t[:, :, :], in_=zfa_d[:, :, :])
            wu = st.tile([1, 8], F32, tag="wu")
            nc.vector.memset(wu[:, :], 0.0)
            nc.scalar.activation(out=wu[:, :], in_=wu[:, :], func=ACT.Exp, scale=1.0)
            zfk_t = keys.tile([32, 2, N], F8, tag="zfk")
            nc.sync.dma_start(out=zfk_t[:, :, 0:N // 2], in_=zfk_d[:, :, 0:N // 2])
            zca_t = anch.tile([64, 2, A], F8, tag="zca")
            nc.sync.dma_start(out=zca_t[:, :, :], in_=zca_d[:, :, :])
            zck_t = keys.tile([64, 2, N], F8, tag="zck")
            nc.sync.dma_start(out=zck_t[:, :, 0:N // 2], in_=zck_d[:, :, 0:N // 2])
            nc.sync.dma_start(out=zfk_t[:, :, N // 2:N], in_=zfk_d[:, :, N // 2:N])
            nc.sync.dma_start(out=zck_t[:, :, N // 2:N], in_=zck_d[:, :, N // 2:N])

            pr = [ps.tile([128, G], F32, tag=f"pr{i}", name=f"pr{i}")
                  for i in range(4)]

            for ab in range(nblk):
                stt = st.tile([128, NSTAT], F32, tag="stt")
                dummy = st.tile([128, G], BF16, tag="dummy")
                for oi, (head, g) in enumerate(order):
                    region = pr[oi % 4]
                    lhsT = (zfa_t if head == "f" else zca_t)[:, :, ab * 128:(ab + 1) * 128]
                    keys_t = zfk_t if head == "f" else zck_t
                    gp = by_group[(head, g)]
                    kmax = max(p["c1"] for p in gp)
                    for kk in range(int(math.ceil(kmax / 512))):
                        nc.tensor.matmul(
                            out=region[:, kk * 512:(kk + 1) * 512],
                            lhsT=lhsT,
                            rhs=keys_t[:, :, g * G + kk * 512:g * G + (kk + 1) * 512],
                            start=True, stop=True, perf_mode=DR)
                    # consumers: ACT pieces first, then Pool/DVE mains, then sums
                    i16 = None
                    if any(p["eng"] in "PD" for p in gp):
                        i16 = sch.tile([128, G], I16, tag=f"i16{oi % 4}")
                    for p in gp:
                        if p["eng"] == "A":
                            nc.scalar.activation(
                                out=region[:, p["c0"]:p["c1"]],
                                in_=region[:, p["c0"]:p["c1"]],
                                func=ACT.Exp, scale=act_scale,
                                accum_out=stt[:, p["stat"]:p["stat"] + 1])
                    # Pool span, then DVE span (each may be split by class cuts;
                    # emit one engine op per contiguous engine span)
                    for eng, engine in (("P", nc.gpsimd), ("D", nc.vector)):
                        sp = [p for p in gp if p["eng"] == eng]
                        if not sp:
                            continue
                        c0 = min(p["c0"] for p in sp)
                        c1 = max(p["c1"] for p in sp)
                        engine.tensor_scalar(
                            out=i16[:, c0:c1], in0=region[:, c0:c1],
                            scalar1=sch_c, scalar2=sch_b,
                            op0=ALU.mult, op1=ALU.add)
                    for p in gp:
                        if p["eng"] in "PD":
                            nc.vector.tensor_scalar(
                                out=dummy[:, p["c0"]:p["c1"]],
                                in0=i16[:, p["c0"]:p["c1"]].bitcast(BF16),
                                scalar1=1.0, scalar2=0.0,
                                op0=ALU.mult, op1=ALU.add,
                                accum_out=stt[:, p["stat"]:p["stat"] + 1])
                nc.sync.dma_start(out=st_d[ab, :, :], in_=stt[:, :])
    nc.compile()
    return nc


def _run(nc, in_maps, out_names):
    import time as _time

    if os.environ.get("CC_BASS_SIM") == "1":
        from concourse import bass_interp

        ncores = int(os.environ.get("CC_BASS_SIM_CORES", str(N_CORES)))
        results = []
        for m in range(ncores):
            sim = bass_interp.CoreSim(nc, core_id=m)
            for k, v in in_maps[m].items():
                sim.tensor(k)[:] = v
            if nc.partition_id_tensor is not None:
                sim.tensor(nc.partition_id_tensor.name)[:] = np.array(
                    [[m]], dtype=np.uint32)
            sim.simulate()
            results.append(
                {name: np.array(sim.mem_tensor(name)) for name in out_names})
        while len(results) < N_CORES:
            results.append(results[-1])
        return results
    t0 = _time.monotonic()
    res = run_bass_kernel_spmd(nc, in_maps, core_ids=list(range(N_CORES)))
    LAST_TIMES.append(_time.monotonic() - t0)
    LAST_RESULTS.append(res)
    return res.results


def kernel(**inputs):
    global LAST_RESULTS, LAST_TIMES
    LAST_RESULTS = []
    LAST_TIMES = []

    roi = np.ascontiguousarray(np.asarray(inputs["roi_feats"], dtype=np.float32))
    labels = np.asarray(inputs["labels"]).astype(np.int64)
    ious = np.asarray(inputs["ious"], dtype=np.float32)
    w1f = np.asarray(inputs["w1f"], dtype=np.float64)
    b1f = np.asarray(inputs["b1f"], dtype=np.float64)
    w2f = np.asarray(inputs["w2f"], dtype=np.float64)
    b2f = np.asarray(inputs["b2f"], dtype=np.float64)
    w1c = np.asarray(inputs["w1c"], dtype=np.float64)
    b1c = np.asarray(inputs["b1c"], dtype=np.float64)
    w2c = np.asarray(inputs["w2c"], dtype=np.float64)
    b2c = np.asarray(inputs["b2c"], dtype=np.float64)
    assert roi.shape == (N, C)

    ign = labels == -1
    fg = (labels > 0) & ~ign
    bg = (labels == 0) & ~ign
    anc = fg & (ious > IOU_THRESHOLD)

    perm = np.concatenate(
        [np.where(anc)[0], np.where(fg & ~anc)[0], np.where(bg)[0], np.where(ign)[0]])
    n_A = int(anc.sum())
    n_fg = int(fg.sum())
    n_valid = n_fg + int(bg.sum())
    if n_A == 0:
        return np.zeros(2, dtype=np.float32)

    x_s = roi[perm]
    labels_s = labels[perm]
    ious_s = ious[perm].astype(np.float64)

    # ---------------- launch 1: MLP ----------------
    w1_all = np.concatenate([w1f, w1c], axis=0)
    b1_all = np.concatenate([b1f, b1c], axis=0) * W1SCALE
    w1_q = (w1_all * W1SCALE).astype(NP8)
    x_q = x_s.astype(NP8)

    h_probe = np.maximum(
        x_q[:256].astype(np.float32) @ w1_q.astype(np.float32).T
        + b1_all.astype(np.float32), 0)
    layer2_fp8 = bool(h_probe.max() < 200.0)

    mlp_key = ("mlp", layer2_fp8)
    if mlp_key not in _NC_CACHE:
        _NC_CACHE[mlp_key] = _build_mlp_nc(layer2_fp8)
    nc1 = _NC_CACHE[mlp_key]

    KC4 = C // 256
    R = N // N_CORES
    w1_dr = np.ascontiguousarray(
        w1_q.T.reshape(KC4, 2, 128, HID2).transpose(2, 0, 1, 3))
    w2_all = np.concatenate([w2f, w2c], axis=0)
    w2dt = NP8 if layer2_fp8 else NPBF
    w2_dr = np.ascontiguousarray(
        w2_all.T.reshape(2, 128, DF + DC).transpose(1, 0, 2)).astype(w2dt)
    b1_dr = np.ascontiguousarray(
        b1_all.reshape(HID2 // 128, 128).T).astype(np.float32)

    xT_q = np.ascontiguousarray(x_q.T)
    shared1 = {"w1": w1_dr, "w2": w2_dr, "b1": b1_dr}
    in_maps1 = []
    for m in range(N_CORES):
        xm = xT_q[:, m * R:(m + 1) * R]
        x_dr = np.ascontiguousarray(
            xm.reshape(KC4, 2, 128, R).transpose(2, 0, 1, 3))
        in_maps1.append({"x": x_dr, **shared1})
    res1 = _run(nc1, in_maps1, ["zf", "zc"])

    zfT_raw = np.concatenate([r["zf"].astype(np.float64) for r in res1], axis=1)
    zcT_raw = np.concatenate([r["zc"].astype(np.float64) for r in res1], axis=1)

    def _normalize(zT_raw, b2):
        z = zT_raw.T + b2[None, :] * W1SCALE
        nrm = np.sqrt(np.sum(z * z, axis=1, keepdims=True)) / W1SCALE
        return z / W1SCALE / np.maximum(nrm, EPS)

    zfn = _normalize(zfT_raw, b2f)
    zcn = _normalize(zcT_raw, b2c)

    zfq = (zfn * ZSCALE).astype(NP8)
    zcq = (zcn * ZSCALE).astype(NP8)
    zfq64 = zfq.astype(np.float64)
    zcq64 = zcq.astype(np.float64)

    # ---------------- launch 2: sims ----------------
    nblk = max(1, math.ceil(math.ceil(n_A / N_CORES) / 128))
    A_pc = nblk * 128
    # ACT extension into cls G1 to balance engines (rounded to 16)
    act_ext = 966 // 16 * 16
    sim_key = ("sim", n_fg, n_valid, nblk, act_ext)
    if sim_key not in _NC_CACHE:
        _NC_CACHE[sim_key] = _build_sim_nc(n_fg, n_valid, nblk, act_ext)
    nc2 = _NC_CACHE[sim_key]
    pieces, _ = _sim_piece_table(n_fg, n_valid, act_ext)

    def _dr(zq_cols):
        d = zq_cols.shape[0]
        return np.ascontiguousarray(zq_cols.reshape(2, d // 2, -1).transpose(1, 0, 2))

    zfqT = np.ascontiguousarray(zfq.T)   # [DF, N]
    zcqT = np.ascontiguousarray(zcq.T)   # [DC, N]
    in_maps2 = []
    for m in range(N_CORES):
        lo = min(m * A_pc, n_A)
        hi = min((m + 1) * A_pc, n_A)
        # local key order: own anchor window first
        local = np.concatenate([
            np.arange(lo, hi),
            np.arange(0, lo),
            np.arange(hi, N),
        ])
        aidx = np.minimum(np.arange(m * A_pc, (m + 1) * A_pc), n_A - 1)
        in_maps2.append({
            "zfk": _dr(zfqT[:, local]),
            "zck": _dr(zcqT[:, local]),
            "zfa": _dr(zfqT[:, aidx]),
            "zca": _dr(zcqT[:, aidx]),
        })
    res2 = _run(nc2, in_maps2, ["stats"])

    NSTAT = len(pieces)
    stats = np.stack([r["stats"].reshape(A_pc, NSTAT) for r in res2], axis=0)
    stats = stats.astype(np.float64)      # [cores, A_pc, NSTAT]

    # ---------------- host: final losses in float64 ----------------
    act_scale = 1.0 / (ZSCALE * ZSCALE * TAU)
    # piece -> class membership (per-core local column space; class sections
    # are preserved by the local reordering, so boundaries are global)
    numer_cols = [p["stat"] for p in pieces
                  if p["head"] == "f" and p["g"] * G + p["c1"] <= n_fg]
    denom_cols = [p["stat"] for p in pieces if p["head"] == "f"]
    denc_cols = [p["stat"] for p in pieces if p["head"] == "c"]

    out_rows = np.empty((n_A, NSTAT), dtype=np.float64)
    for m in range(N_CORES):
        lo = m * A_pc
        hi = min((m + 1) * A_pc, n_A)
        if hi > lo:
            out_rows[lo:hi] = stats[m, : hi - lo]
    stats = out_rows                      # [n_A, NSTAT]

    w_a = ious_s[:n_A]
    sdot_f = np.einsum("nd,nd->n", zfq64[:n_A], zfq64[:n_A])
    sdot_c = np.einsum("nd,nd->n", zcq64[:n_A], zcq64[:n_A])
    selfexp_f = np.exp(sdot_f * act_scale)
    selfexp_c = np.exp(sdot_c * act_scale)

    numer = stats[:, numer_cols].sum(1) - selfexp_f
    denom = stats[:, denom_cols].sum(1) - selfexp_f
    denom_c = stats[:, denc_cols].sum(1) - selfexp_c

    if n_fg - 1 > 0:
        li = -np.log((numer + EPS) / (denom + EPS))
        loss_fg = np.sum(li * w_a) / (np.sum(w_a) + EPS)
    else:
        loss_fg = 0.0

    lab_valid = labels_s[:n_valid]
    cnt = np.bincount(lab_valid, minlength=21)
    S = np.zeros((21, DC), dtype=np.float64)
    np.add.at(S, lab_valid, zcn[:n_valid])
    c_a = labels_s[:n_A]
    n_pos = (cnt[c_a] - 1).astype(np.float64)
    denom_log = np.log(np.maximum(denom_c, 1e-300))
    zca64 = zcn[:n_A]
    sum_pos = (np.einsum("nd,nd->n", zca64, S[c_a])
               - np.einsum("nd,nd->n", zca64, zca64)) / TAU
    li_c = -(sum_pos - n_pos * denom_log) / np.maximum(n_pos, 1.0)
    valid_c = n_pos > 0
    num2 = np.sum(np.where(valid_c, li_c * w_a, 0.0))
    den2 = np.sum(np.where(valid_c, w_a, 0.0))
    loss_cls = num2 / (den2 + EPS12)

    return np.stack([loss_fg, loss_cls]).astype(np.float32)
